# revision 28
# baseline (speedup 1.0000x reference)
"""Fused pre-norm decoder layer (RMSNorm + GQA causal attention w/ RoPE +
RMSNorm + SwiGLU MLP) on 8 Trainium2 NeuronCores.

Sharding: sequence-parallel with folded stripe pairs — core c owns row stripes
{c, 15-c} (128 rows each) so causal attention work is balanced; the MLP is
tensor-parallel (w1/w3 column-split, w2 row-split). Cross-core comms:
AllGather of roped K^T + V (bf16), AllGather of the transposed normed hidden
states (bf16), and a 4-way chunked ReduceScatter of the MLP partial outputs
(bf16) overlapped with the w2 matmuls. Matmuls in bf16, residuals in fp32.

Self-contained: hardcodes the reference shapes
(B=1, N=2048, DIM=2048, HQ=16, HK=4, HD=128, F=8192).
"""
import numpy as np
import ml_dtypes

import concourse.bass as bass
import concourse.mybir as mybir
import concourse.tile as tile
from concourse import bacc
from concourse.bass_utils import run_bass_kernel_spmd
from concourse.masks import make_identity

F32 = mybir.dt.float32
BF16 = mybir.dt.bfloat16
AF = mybir.ActivationFunctionType
ALU = mybir.AluOpType
BF = ml_dtypes.bfloat16

DIM = 2048
HQ = 16            # query heads
HK = 4             # kv heads
HD = 128           # head dim
KV = HD * HK       # 512
N = 2048           # sequence length
FF = 4 * DIM       # 8192 mlp hidden
EPS = 1e-6
ROPE_BASE = 10000.0
SCALE = HD ** -0.5

NCORES = 8
RG = [list(range(NCORES))]
NCH = N // 128       # 16 sequence chunks
NIC = DIM // 128     # 16 feature chunks
FSH = FF // NCORES   # 1024 mlp hidden per core
FSC = FSH // 128     # 8 f-chunks per core
NEG = -1e30
DEBUG = False

# core c owns stripes (c, 15-c); local rows = [stripe_c | stripe_{15-c}]
# global s-chunk j lives on core own(j), slot slot(j):
def _owner(j):
    return (j, 0) if j < NCH // 2 else (NCH - 1 - j, 1)


def _build_kernel():
    nc = bacc.Bacc(None, target_bir_lowering=False)

    x_rows = nc.dram_tensor("x_rows", [2, 128, DIM], F32, kind="ExternalInput")
    rtab = nc.dram_tensor("rtab", [2, 2, 128, 256], F32, kind="ExternalInput")
    masks = nc.dram_tensor("masks", [6, 128, 512], BF16, kind="ExternalInput")
    biases = nc.dram_tensor("biases", [2, 3072], BF16, kind="ExternalInput")
    wqkvT = nc.dram_tensor("wqkvT", [DIM, 3072], BF16, kind="ExternalInput")
    woT = nc.dram_tensor("woT", [DIM, DIM], BF16, kind="ExternalInput")
    w1S = nc.dram_tensor("w1S", [FSC, 128, DIM], BF16, kind="ExternalInput")
    w3S = nc.dram_tensor("w3S", [FSC, 128, DIM], BF16, kind="ExternalInput")
    w2T = nc.dram_tensor("w2T", [FSH, DIM], BF16, kind="ExternalInput")
    out_ext = nc.dram_tensor("out", [2, 128, DIM], F32, kind="ExternalOutput")
    dbg = {}
    if DEBUG:
        for nm, shp, dt in [("dbg_qkv", [128, 2, 3072], BF16),
                            ("dbg_attn", [128, 2, DIM], BF16),
                            ("dbg_h", [128, 2, DIM], F32),
                            ("dbg_xn", [128, 2, DIM], BF16),
                            ("dbg_xnT", [128, NIC, 256], BF16)]:
            dbg[nm] = nc.dram_tensor(nm, shp, dt, kind="ExternalOutput")

    with tile.TileContext(nc) as tc:
        _body(nc, tc, x_rows, rtab, masks, biases,
              wqkvT, woT, w1S, w3S, w2T, out_ext, dbg)
    nc.compile()
    return nc


def _rope_psum(nc, rp, rtab_sb, pcur, sl, dst):
    """rope a [128, 512] psum tile (4 head-blocks) into dst [128, 512] bf16."""
    AFv = mybir.ActivationFunctionType
    pv = pcur.rearrange("p (h t) -> p h t", t=128)
    cosT = rtab_sb[:, sl, 0, :].rearrange("p (h t) -> p h t", t=64)
    sinT = rtab_sb[:, sl, 1, :].rearrange("p (h t) -> p h t", t=64)
    t1 = rp.tile([128, 4, 64], F32, name="t1", tag="t1")
    t2 = rp.tile([128, 4, 64], F32, name="t2", tag="t2")
    t3 = rp.tile([128, 4, 64], F32, name="t3", tag="t3")
    t4 = rp.tile([128, 4, 64], F32, name="t4", tag="t4")
    nc.vector.tensor_mul(t1[:], pv[:, :, 0:64], cosT)
    nc.vector.tensor_mul(t2[:], pv[:, :, 64:128], sinT)
    nc.vector.tensor_mul(t3[:], pv[:, :, 0:64], sinT)
    nc.vector.tensor_mul(t4[:], pv[:, :, 64:128], cosT)
    dstv = dst.rearrange("p (h t) -> p h t", t=128)
    nc.vector.tensor_sub(dstv[:, :, 0:64], t1[:], t2[:])
    nc.vector.tensor_add(dstv[:, :, 64:128], t3[:], t4[:])


def _rmsnorm_to(nc, pool, out_bf, x_sb, slot, eps_tile, scratch):
    """out_bf[:, slot, :] = rmsnorm(x_sb[:, slot, :]) cast bf16.
    scratch: any writable [128, DIM] f32 AP whose contents may be clobbered."""
    ssq = pool.tile([128, 1], F32, name="ssq", tag="ssq")
    nc.scalar.activation(scratch, x_sb[:, slot, :], AF.Square, accum_out=ssq[:])
    rms = pool.tile([128, 1], F32, name="rms", tag="rms")
    nc.scalar.activation(rms[:], ssq[:], AF.Sqrt, bias=eps_tile[:], scale=1.0 / DIM)
    rinv = pool.tile([128, 1], F32, name="rinv", tag="rinv")
    nc.vector.reciprocal(rinv[:], rms[:])
    nc.vector.tensor_scalar_mul(out_bf[:, slot, :], x_sb[:, slot, :], rinv[:])


def _transpose_2x16(nc, sb, ps, dst, src, ident, tag):
    """src [128, 2, 2048] bf16 row-major -> dst [128, 16, 256] bf16 (feature-major).
    dst[:, ic, s*128:(s+1)*128] = src[:, s, ic*128:(ic+1)*128].T
    """
    for s in range(2):
        for ic in range(NIC):
            tp = ps.tile([128, 128], BF16, name=f"tp_{tag}", tag=f"tp_{tag}")
            nc.tensor.transpose(tp[:], src[:, s, ic * 128:(ic + 1) * 128], ident[:])
            nc.vector.tensor_copy(dst[:, ic, s * 128:(s + 1) * 128], tp[:])


def _body(nc, tc, x_rows, rtab, masks, biases,
          wqkvT, woT, w1S, w3S, w2T, out_ext, dbg={}):
    import contextlib
    ctx = contextlib.ExitStack()
    with ctx:
        const = ctx.enter_context(tc.tile_pool(name="const", bufs=1))
        persist = ctx.enter_context(tc.tile_pool(name="persist", bufs=1))
        dram = ctx.enter_context(tc.tile_pool(name="dram", bufs=1, space="DRAM"))
        small = ctx.enter_context(tc.tile_pool(name="small", bufs=4))

        ident = const.tile([128, 128], BF16)
        make_identity(nc, ident)
        eps_tile = const.tile([128, 1], F32)
        nc.gpsimd.memset(eps_tile[:], EPS)
        ones_bf = const.tile([1, 512], BF16)
        nc.gpsimd.memset(ones_bf[:], 1.0)
        bias_sb = None  # allocated in attention pool below

        # DRAM comm buffers
        agk_in = dram.tile([HK * 128 * 256], BF16)
        agk_out = dram.tile([NCORES, HK * 128 * 256], BF16,
                            addr_space="Shared")
        agv_in = dram.tile([2 * 128 * KV], BF16)
        agv_out = dram.tile([NCORES, 2 * 128 * KV], BF16,
                            addr_space="Shared")
        agx_in = dram.tile([2, NIC * 128 * 128], BF16)
        agx_out0 = dram.tile([NCORES, NIC * 128 * 128], BF16,
                             addr_space="Shared")
        agx_out1 = dram.tile([NCORES, NIC * 128 * 128], BF16,
                             addr_space="Shared")
        agx_outs = [agx_out0, agx_out1]
        rs_in = dram.tile([4, 512, DIM], BF16)
        rs_out = dram.tile([4, 64, DIM], BF16)

        # persistent SBUF
        h_sb = persist.tile([128, 2, DIM], F32)       # post-attention residual
        x2nT = persist.tile([128, NIC, 256], BF16)

        # attention-phase pool: closed before the MLP pool allocates
        att_ctx = contextlib.ExitStack()
        ph1 = att_ctx.enter_context(tc.tile_pool(name="ph1", bufs=1))
        qkv_ctx = contextlib.ExitStack()
        qkvp = qkv_ctx.enter_context(tc.tile_pool(name="qkvp", bufs=1))
        rtab_sb = qkvp.tile([128, 2, 2, 256], F32)
        nc.gpsimd.dma_start(rtab_sb[:], rtab.rearrange("s c p t -> p s c t"))
        x_sb = ph1.tile([128, 2, DIM], F32)           # own rows [slotL | slotH]
        for s in range(2):
            eng = nc.sync if s == 0 else nc.scalar
            eng.dma_start(x_sb[:, s, :], x_rows[s])
        # K/V projection weights fully resident so the K AllGather can launch
        # as early as possible (critical path of the whole kernel)
        wkv_ctx = contextlib.ExitStack()
        wkvres = wkv_ctx.enter_context(tc.tile_pool(name="wkvres", bufs=1))
        wkv_sb = wkvres.tile([128, NIC, 1024], BF16)
        _wkv_engs = [nc.gpsimd, nc.scalar, nc.sync, nc.scalar]
        for hf in range(4):
            _wkv_engs[hf].dma_start(
                wkv_sb[:, hf * 4:(hf + 1) * 4, :],
                wqkvT[:, 0:1024].rearrange(
                    "(i p) c -> p i c", p=128)[:, hf * 4:(hf + 1) * 4, :])
        # PE warmup: DMA-independent matmuls raise HAM to K=8/8 while the
        # prologue DMAs/norm run; result sunk to DRAM to stay live.
        warm_sink = dram.tile([128, 1], F32)
        with (
            tc.tile_pool(name="warmp", bufs=1, space="PSUM") as warmp,
            tc.tile_pool(name="warms", bufs=1) as warms,
        ):
            wrm = warms.tile([128, 512], BF16)
            nc.gpsimd.memset(wrm[:], 1.0)
            wps = warmp.tile([128, 512], F32)
            for wi in range(8):
                nc.tensor.matmul(wps[:], ident[:], wrm[:],
                                 start=True, stop=True)
            wsb = warms.tile([128, 1], F32)
            nc.vector.tensor_copy(wsb[:], wps[:, 0:1])
            nc.sync.dma_start(warm_sink[:], wsb[:])
        bias_sb = ph1.tile([1, 2, 3072], BF16)
        for i in range(2):
            nc.sync.dma_start(bias_sb[0:1, i, :], biases[i:i + 1, :])
        mask_sb = ph1.tile([128, 6, 512], BF16)
        nc.sync.dma_start(mask_sb[:], masks.rearrange("k p q -> p k q"))
        xn = qkvp.tile([128, 2, DIM], BF16)
        for s in range(2):
            _rmsnorm_to(nc, small, xn, x_sb, s, eps_tile, h_sb[:, s, :])
        xnT = qkvp.tile([128, NIC, 256], BF16)

        # ===== phase 2a: K/V projections + rope-k + early AllGathers =====
        # qkv_rows[:, slot, 0:2048]=roped q, [2048:2560]=roped k, [2560:3072]=v
        # wqkvT col order: [k(512) | v(512) | q(2048)]
        # xn transposes are interleaved into the K/V matmul loop (transpose for
        # ic runs ~4 iterations ahead of its consuming matmuls).
        qkv_rows = qkvp.tile([128, 2, 3072], BF16)
        kT_own = qkvp.tile([128, HK, 256], BF16)
        q_roped = ph1.tile([128, HQ, 256], BF16)

        def _xn_transpose(ps1, ic):
            for s in range(2):
                tp = ps1.tile([128, 128], BF16, name="tp_xn", tag="tp_xn")
                nc.tensor.transpose(tp[:], xn[:, s, ic * 128:(ic + 1) * 128],
                                    ident[:])
                nc.vector.tensor_copy(xnT[:, ic, s * 128:(s + 1) * 128], tp[:])

        with (
            tc.tile_pool(name="pkv", bufs=1, space="PSUM") as pkv,
            tc.tile_pool(name="rp", bufs=2) as rp,
        ):
            ps = [pkv.tile([128, 512], F32, name=f"pkv{u}", tag=f"pkv{u}")
                  for u in range(4)]          # (oc, slot): oc0=k, oc1=v
            with tc.tile_pool(name="tp1", bufs=3, space="PSUM") as ps1:
                for ic in range(4):
                    _xn_transpose(ps1, ic)
                for ic in range(NIC):
                    if ic + 4 < NIC:
                        _xn_transpose(ps1, ic + 4)
                    for oi in range(2):
                        for sl in range(2):
                            nc.tensor.matmul(
                                ps[oi * 2 + sl][:],
                                xnT[:, ic, sl * 128:(sl + 1) * 128],
                                wkv_sb[:, ic, oi * 512:(oi + 1) * 512],
                                start=(ic == 0), stop=False)
            for oi in range(2):
                for sl in range(2):
                    nc.tensor.matmul(
                        ps[oi * 2 + sl][:], ones_bf[:, 0:128],
                        bias_sb[:, 0, oi * 512:(oi + 1) * 512],
                        start=False, stop=True)
            # k: rope then transpose + ship + AG (K first: attention scores
            # gate on it); v: plain copy, ship + AG second.
            for sl in range(2):
                _rope_psum(nc, rp, rtab_sb, ps[0 + sl],
                           sl, qkv_rows[:, sl, 2048:2560])
            with tc.tile_pool(name="tpk", bufs=2, space="PSUM") as tpk:
                for sl in range(2):
                    for kh in range(HK):
                        tp = tpk.tile([128, 128], BF16, name="tp_k",
                                      tag="tp_k")
                        nc.tensor.transpose(
                            tp[:],
                            qkv_rows[:, sl,
                                     2048 + kh * 128:2048 + (kh + 1) * 128],
                            ident[:])
                        nc.vector.tensor_copy(
                            kT_own[:, kh, sl * 128:(sl + 1) * 128], tp[:])
            nc.sync.dma_start(
                agk_in.rearrange("(k d n) -> d k n", k=HK, d=128),
                kT_own[:])
            nc.gpsimd.collective_compute(
                "AllGather", ALU.bypass, replica_groups=RG,
                ins=[agk_in.opt()], outs=[agk_out.opt()])
            for sl in range(2):
                nc.vector.tensor_copy(qkv_rows[:, sl, 2560:3072],
                                      ps[2 + sl][:])
            nc.sync.dma_start(
                agv_in.rearrange("(t2 t k) -> t t2 k", t2=2, t=128),
                qkv_rows[:, :, 2560:3072])
            nc.gpsimd.collective_compute(
                "AllGather", ALU.bypass, replica_groups=RG,
                ins=[agv_in.opt()], outs=[agv_out.opt()])

        wkv_ctx.close()
        # ===== phase 2b: Q projections + rope + transposes (overlap AGs) ====
        with (
            tc.tile_pool(name="wq", bufs=6) as wqp,
            tc.tile_pool(name="pq", bufs=1, space="PSUM") as pq,
            tc.tile_pool(name="rp2", bufs=2) as rp2,
        ):
            psq = [pq.tile([128, 512], F32, name=f"pq{u}", tag=f"pq{u}")
                   for u in range(8)]         # (oc, slot)
            for ic in range(NIC):
                w_t = wqp.tile([128, 2048], BF16, name="wq_t", tag="wqt")
                eng = nc.sync if ic % 2 == 0 else nc.gpsimd
                eng.dma_start(
                    w_t[:], wqkvT[ic * 128:(ic + 1) * 128, 1024:3072])
                for oi in range(4):
                    for sl in range(2):
                        nc.tensor.matmul(
                            psq[oi * 2 + sl][:],
                            xnT[:, ic, sl * 128:(sl + 1) * 128],
                            w_t[:, oi * 512:(oi + 1) * 512],
                            start=(ic == 0), stop=False)
            for oi in range(4):
                for sl in range(2):
                    nc.tensor.matmul(
                        psq[oi * 2 + sl][:], ones_bf[:, 0:128],
                        bias_sb[:, 0, 1024 + oi * 512:1024 + (oi + 1) * 512],
                        start=False, stop=True)
            for oi in range(4):
                for sl in range(2):
                    _rope_psum(nc, rp2, rtab_sb, psq[oi * 2 + sl],
                               sl, qkv_rows[:, sl, oi * 512:(oi + 1) * 512])
        # transposes: q -> q_roped [d, h, n] (head-ascending: scores gate on
        # low heads first)
        with tc.tile_pool(name="tpq", bufs=3, space="PSUM") as tpq:
            for h in range(HQ):
                for sl in range(2):
                    tp = tpq.tile([128, 128], BF16, name="tp_q", tag="tp_q")
                    nc.tensor.matmul(tp[:], qkv_rows[:, sl, h * 128:(h + 1) * 128],
                                     ident[:], is_transpose=True)
                    nc.vector.tensor_copy(q_roped[:, h, sl * 128:(sl + 1) * 128], tp[:])
        if dbg:
            nc.sync.dma_start(dbg["dbg_qkv"][:], qkv_rows[:])
            nc.sync.dma_start(dbg["dbg_xn"][:], xn[:])
            nc.sync.dma_start(dbg["dbg_xnT"][:], xnT[:])
        qkv_ctx.close()

        # ============ phase 4: gather K/V into SBUF (rank-major layouts) ============
        # kT_full[:, kh, r, slot*128+t] = rank r's K slot cols; unit code indexes
        # via _owner(j) -> (r, slot).  K gathers first (scores gate on them),
        # spread over 3 queues.
        kT_full = ph1.tile([128, HK, NCORES, 256], BF16)
        v_aug = ph1.tile([128, NCORES, 2, HK, 132], BF16)
        kengs = [nc.sync, nc.gpsimd, nc.scalar]
        for r in range(NCORES):
            kengs[r % 3].dma_start(
                kT_full[:, :, r, :],
                agk_out[r].rearrange("(k d n) -> d k n", k=HK, d=128))
        nc.gpsimd.memset(v_aug[:, :, :, :, 128:129], 1.0)
        for r in range(NCORES):
            vsrc = agv_out[r].rearrange(
                "(t2 t k d) -> t t2 k d", t2=2, t=128, k=HK)
            for sl2 in range(2):
                kengs[r % 3].dma_start(v_aug[:, r, sl2, :, 0:128],
                                       vsrc[:, sl2, :, :])

        # ============ phase 5: attention ============
        # units 0..7 (pairs of s-chunks vs both stripes) packed 2-per-psum-bank;
        # units 8..15 (H-stripe only) packed 4-per-bank. One stt+exp per bank.
        # Score phases are hoisted AV_DEPTH heads ahead of AV phases so the
        # PE keeps scoring while the V AllGather is still in flight, and the
        # vector/scalar mask+exp chain pipelines ahead of the AV matmuls.
        attn = ph1.tile([128, 2, DIM], BF16)     # row-major attn out (normalized)
        attnT = ph1.tile([128, NIC, 256], BF16)
        AV_DEPTH = 4
        with (
            tc.tile_pool(name="ps_sc", bufs=3, space="PSUM") as ps_sc,
            tc.tile_pool(name="ps_av", bufs=2, space="PSUM") as ps_av,
            tc.tile_pool(name="tp_at", bufs=1, space="PSUM") as tp_at,
            tc.tile_pool(name="att_sb", bufs=6 * (AV_DEPTH + 1)) as att_sbp,
            tc.tile_pool(name="att_tmp", bufs=3) as att_tmp,
        ):
            def scores_phase(h):
                kh = h % HK
                att_tiles = []
                for pair in range(4):            # units 2*pair, 2*pair+1
                    sc = ps_sc.tile([128, 512], F32, name="sc", tag="sc")
                    for u in range(2):
                        k = pair * 2 + u
                        rk, sk = _owner(k)
                        nc.tensor.matmul(
                            sc[:, u * 256:(u + 1) * 256],
                            kT_full[:, kh, rk, sk * 128:(sk + 1) * 128],
                            q_roped[:, h, :], start=True, stop=True)
                    tmp = att_tmp.tile([128, 512], BF16, name="mtmp", tag="mtmp")
                    nc.scalar.activation(tmp[:], sc[:], AF.Exp, scale=SCALE)
                    att = att_sbp.tile([128, 512], BF16, name="attP", tag="attP")
                    nc.vector.tensor_mul(att[:], tmp[:], mask_sb[:, pair, :])
                    att_tiles.append(att)
                for quad in range(2):            # units 8..11, 12..15 (H only)
                    sc = ps_sc.tile([128, 512], F32, name="sc", tag="sc")
                    for u in range(4):
                        k = 8 + quad * 4 + u
                        rk, sk = _owner(k)
                        nc.tensor.matmul(
                            sc[:, u * 128:(u + 1) * 128],
                            kT_full[:, kh, rk, sk * 128:(sk + 1) * 128],
                            q_roped[:, h, 128:256], start=True, stop=True)
                    tmp = att_tmp.tile([128, 512], BF16, name="mtmp", tag="mtmp")
                    nc.scalar.activation(tmp[:], sc[:], AF.Exp, scale=SCALE)
                    att = att_sbp.tile([128, 512], BF16, name="attP", tag="attP")
                    nc.vector.tensor_mul(att[:], tmp[:], mask_sb[:, 4 + quad, :])
                    att_tiles.append(att)
                return att_tiles

            def av_phase(h, att_tiles):
                kh = h % HK
                av = [ps_av.tile([128, 132], F32, name=f"av{s}", tag=f"av{s}")
                      for s in range(2)]
                for pair in range(4):
                    att = att_tiles[pair]
                    for u in range(2):
                        k = pair * 2 + u
                        rk, sk = _owner(k)
                        vap = v_aug[:, rk, sk, kh, 0:129]
                        nc.tensor.matmul(
                            av[0][:, 0:129], att[:, u * 256:u * 256 + 128],
                            vap, start=(k == 0), stop=(k == 7))
                        nc.tensor.matmul(
                            av[1][:, 0:129], att[:, u * 256 + 128:u * 256 + 256],
                            vap, start=(k == 0), stop=False)
                for quad in range(2):
                    att = att_tiles[4 + quad]
                    for u in range(4):
                        k = 8 + quad * 4 + u
                        rk, sk = _owner(k)
                        nc.tensor.matmul(
                            av[1][:, 0:129], att[:, u * 128:(u + 1) * 128],
                            v_aug[:, rk, sk, kh, 0:129],
                            start=False, stop=(k == NCH - 1))
                # normalize by denominator (col 128), then transpose this
                # head's column into attnT right away
                for s in range(2):
                    rd = small.tile([128, 1], F32, name="rd", tag="rd")
                    nc.vector.reciprocal(rd[:], av[s][:, 128:129])
                    nc.vector.tensor_scalar_mul(
                        attn[:, s, h * 128:(h + 1) * 128], av[s][:, 0:128], rd[:])
                for s in range(2):
                    tp = tp_at.tile([128, 128], BF16, name="tp_a", tag="tp_a")
                    nc.tensor.transpose(
                        tp[:], attn[:, s, h * 128:(h + 1) * 128], ident[:])
                    nc.vector.tensor_copy(
                        attnT[:, h, s * 128:(s + 1) * 128], tp[:])

            pend = []
            for h in range(HQ):
                pend.append((h, scores_phase(h)))
                if len(pend) > AV_DEPTH:
                    av_phase(*pend.pop(0))
            for item in pend:
                av_phase(*item)

        if dbg:
            nc.sync.dma_start(dbg["dbg_attn"][:], attn[:])

        # ============ phase 6: output projection + residual ============
        with (
            tc.tile_pool(name="wo", bufs=6) as wop,
            tc.tile_pool(name="po", bufs=1, space="PSUM") as po,
        ):
            pso = [po.tile([128, 512], F32, name=f"pso{i}", tag=f"pso{i}")
                   for i in range(8)]      # (slot, oc4)
            for ic in range(NIC):
                wo_t = wop.tile([128, DIM], BF16, name="wo_t", tag="wo")
                eng = nc.sync if ic % 2 == 0 else nc.gpsimd
                eng.dma_start(wo_t[:], woT[ic * 128:(ic + 1) * 128, :])
                for s in range(2):
                    for oc in range(4):
                        nc.tensor.matmul(
                            pso[s * 4 + oc][:],
                            attnT[:, ic, s * 128:(s + 1) * 128],
                            wo_t[:, oc * 512:(oc + 1) * 512],
                            start=(ic == 0), stop=False)
            for s in range(2):
                for oc in range(4):
                    nc.tensor.matmul(
                        pso[s * 4 + oc][:], ones_bf[:, 0:128],
                        bias_sb[:, 1, oc * 512:(oc + 1) * 512],
                        start=False, stop=True)
            for s in range(2):
                for oc in range(4):
                    nc.vector.tensor_add(
                        h_sb[:, s, oc * 512:(oc + 1) * 512],
                        pso[s * 4 + oc][:], x_sb[:, s, oc * 512:(oc + 1) * 512])

        if dbg:
            nc.sync.dma_start(dbg["dbg_h"].rearrange("p s d -> p s d"), h_sb[:])
        # ============ phase 7: norm2 + transpose + per-slot group AG ============
        # AG for slot L ships as soon as its 16 transposes land so the MLP's
        # L-half can start while slot H is still being normed/gathered.
        x2n = ph1.tile([128, 2, DIM], BF16)
        with tc.tile_pool(name="tp3", bufs=3, space="PSUM") as ps3:
            for s in range(2):
                _rmsnorm_to(nc, small, x2n, h_sb, s, eps_tile, x_sb[:, s, :])
                for ic in range(NIC):
                    tp = ps3.tile([128, 128], BF16, name="tp_x2", tag="tp_x2")
                    nc.tensor.transpose(
                        tp[:], x2n[:, s, ic * 128:(ic + 1) * 128], ident[:])
                    nc.vector.tensor_copy(
                        x2nT[:, ic, s * 128:(s + 1) * 128], tp[:])
                eng = nc.sync if s == 0 else nc.scalar
                eng.dma_start(
                    agx_in[s].rearrange("(i p t) -> p i t", i=NIC, p=128),
                    x2nT[:, :, s * 128:(s + 1) * 128])
                nc.gpsimd.collective_compute(
                    "AllGather", ALU.bypass, replica_groups=RG,
                    ins=[agx_in[s].opt()], outs=[agx_outs[s].opt()])

        # ============ phase 8: MLP (TP, FF/8) with chunked RS ============
        # g-chunks 0,1 read only slot-L columns (nbase < 128) so they gate on
        # the slot-L AllGather alone; slot-H arrives while g=0,1 compute.
        att_ctx.close()
        mlpw = ctx.enter_context(tc.tile_pool(name="mlpw", bufs=1, side="right"))
        w2_sb = mlpw.tile([128, FSC, DIM], BF16)
        for hf in range(2):
            eng = nc.gpsimd if hf == 0 else nc.scalar
            eng.dma_start(
                w2_sb[:, hf * 4:(hf + 1) * 4, :],
                w2T.rearrange("(f p) o -> p f o", p=128)[:, hf * 4:(hf + 1) * 4, :])
        mlp = ctx.enter_context(tc.tile_pool(name="mlp", bufs=1, side="right"))
        x2nT_full = mlp.tile([128, NIC, N], BF16)
        for s in range(2):
            for r in range(NCORES):
                kengs[r % 3].dma_start(
                    x2nT_full[:, :, r * 256 + s * 128:r * 256 + (s + 1) * 128],
                    agx_outs[s][r].rearrange("(i p t) -> p i t", i=NIC, p=128))

        w13p = mlpw
        with (
            tc.tile_pool(name="ps_y", bufs=2, space="PSUM") as ps_y,
            tc.tile_pool(name="h2p", bufs=2) as h2p,
            tc.tile_pool(name="ps_w2", bufs=2, space="PSUM") as ps_w2,
            tc.tile_pool(name="rs_sb", bufs=3) as rs_sbp,
        ):
            for g in range(4):             # n-super-chunk = P2 block g
                # rhs columns: P1 cols {r*256 + (g//2)*128 + (g%2)*64 .. +64}
                nbase = (g // 2) * 128 + (g % 2) * 64
                h2T = h2p.tile([128, FSC, 512], BF16, name="h2T", tag="h2T")
                for f in range(FSC):
                    w1_t = w13p.tile([128, NIC, 128], BF16, name="w1_t", tag="w1", bufs=3)
                    nc.sync.dma_start(
                        w1_t.rearrange("p i f -> p (i f)"), w1S[f])
                    w3_t = w13p.tile([128, NIC, 128], BF16, name="w3_t", tag="w3", bufs=3)
                    nc.gpsimd.dma_start(
                        w3_t.rearrange("p i f -> p (i f)"), w3S[f])
                    y1 = ps_y.tile([128, 512], F32, name="y1", tag="y1")
                    y3 = ps_y.tile([128, 512], F32, name="y3", tag="y3")
                    for ic in range(NIC):
                        rhs = x2nT_full[:, ic, :].rearrange(
                            "p (r t) -> p r t", t=256)[:, :, nbase:nbase + 64]
                        nc.tensor.matmul(y1[:], w1_t[:, ic, :], rhs,
                                         start=(ic == 0), stop=(ic == NIC - 1))
                        nc.tensor.matmul(y3[:], w3_t[:, ic, :], rhs,
                                         start=(ic == 0), stop=(ic == NIC - 1))
                    sg = rs_sbp.tile([128, 512], BF16, name="sg", tag="sg")
                    nc.scalar.activation(sg[:], y1[:], AF.Sigmoid)
                    sil = rs_sbp.tile([128, 512], F32, name="sil", tag="sil")
                    nc.vector.scalar_tensor_tensor(
                        sil[:], y1[:], 1.0, sg[:], op0=ALU.mult, op1=ALU.mult)
                    nc.vector.tensor_mul(h2T[:, f, :], sil[:], y3[:])
                # w2: out partial rows for P2 block g
                for q in range(4):         # 128-row slices within block
                    for oc in range(4):
                        pw = ps_w2.tile([128, 512], F32, name="pw", tag="pw")
                        for f in range(FSC):
                            nc.tensor.matmul(
                                pw[:], h2T[:, f, q * 128:(q + 1) * 128],
                                w2_sb[:, f, oc * 512:(oc + 1) * 512],
                                start=(f == 0), stop=(f == FSC - 1))
                        ob = rs_sbp.tile([128, 512], BF16, name="ob", tag="ob")
                        if (q * 4 + oc) % 2 == 0:
                            nc.vector.tensor_copy(ob[:], pw[:])
                        else:
                            nc.scalar.copy(ob[:], pw[:])
                        nc.sync.dma_start(
                            rs_in[g, q * 128:(q + 1) * 128,
                                  oc * 512:(oc + 1) * 512], ob[:])
                nc.gpsimd.collective_compute(
                    "ReduceScatter", ALU.add, replica_groups=RG,
                    ins=[rs_in[g].opt()], outs=[rs_out[g].opt()])

        # ============ phase 9: final residual + output (per RS chunk) ============
        rs_res = mlp.tile([128, 2, DIM], BF16)
        out_sb = mlp.tile([128, 2, DIM], F32)
        for g in range(4):
            s, half = g // 2, g % 2
            pr = slice(half * 64, (half + 1) * 64)
            eng = nc.sync if g % 2 == 0 else nc.gpsimd
            eng.dma_start(rs_res[pr, s, :], rs_out[g])
            nc.vector.tensor_add(out_sb[pr, s, :], rs_res[pr, s, :], h_sb[pr, s, :])
            eng.dma_start(out_ext[s, pr, :], out_sb[pr, s, :])


# ============================ host side ============================

def _perm(nheads):
    p = []
    for h in range(nheads):
        base = h * HD
        p.extend(range(base, base + HD, 2))
        p.extend(range(base + 1, base + HD, 2))
    return np.array(p)


def _rope_tabs(pos):
    inv = 1.0 / (ROPE_BASE ** (np.arange(0, HD, 2, dtype=np.float32) / HD))
    f = np.outer(pos.astype(np.float32), inv)        # [n, 64]
    return np.cos(f).T.astype(np.float32), np.sin(f).T.astype(np.float32)


def _mask_for(stripe, j):
    """multiplicative mask [128 s, 128 q] for s-chunk j vs q-stripe `stripe`"""
    if j < stripe:
        return np.ones((128, 128), np.float32)
    if j > stripe:
        return np.zeros((128, 128), np.float32)
    i = np.arange(128)
    return np.where(i[:, None] <= i[None, :], 1.0, 0.0).astype(np.float32)


def _wimg(wshard):
    """[1024, 2048] w-shard -> SBUF images [8 f-chunks, 128 part(i%128), 16*128]
    img[f][p, ic*128+t] = w.T[ic*128+p, f*128+t]"""
    wT = wshard.T                      # [2048 i, 1024 f]
    img = wT.reshape(NIC, 128, FSC, 128).transpose(2, 1, 0, 3).reshape(FSC, 128, DIM)
    return np.ascontiguousarray(img).astype(BF)


_CACHED_NC = None


def _get_nc():
    global _CACHED_NC
    if _CACHED_NC is None:
        _CACHED_NC = _build_kernel()
    return _CACHED_NC


def _prep_in_maps(inputs):
    f32 = lambda a: np.ascontiguousarray(np.asarray(a), dtype=np.float32)
    x = f32(inputs["x"])[0]                  # [N, DIM]
    g_attn, g_mlp = f32(inputs["g_attn"]), f32(inputs["g_mlp"])
    pq, pk = _perm(HQ), _perm(HK)
    wq = f32(inputs["wq"])[pq] * g_attn[None, :]
    wk = f32(inputs["wk"])[pk] * g_attn[None, :]
    wv = f32(inputs["wv"]) * g_attn[None, :]
    wo = f32(inputs["wo"])
    w1 = f32(inputs["w1"]) * g_mlp[None, :]
    w3 = f32(inputs["w3"]) * g_mlp[None, :]
    w2 = f32(inputs["w2"])
    biases = np.zeros((2, 3072), np.float32)
    biases[0, 0:KV] = f32(inputs["bk"])[pk]
    biases[0, KV:2 * KV] = f32(inputs["bv"])
    biases[0, 2 * KV:] = f32(inputs["bq"])[pq]
    biases[1, 0:DIM] = f32(inputs["bo"])

    wqkv = np.concatenate([wk, wv, wq], 0)         # [3072, 2048] (k|v|q)
    shared = {
        "wqkvT": np.ascontiguousarray(wqkv.T).astype(BF),
        "woT": np.ascontiguousarray(wo.T).astype(BF),
        "biases": biases.astype(BF),
    }
    in_maps = []
    for c in range(NCORES):
        sl, sh = c, NCH - 1 - c
        pos = np.concatenate([np.arange(sl * 128, (sl + 1) * 128),
                              np.arange(sh * 128, (sh + 1) * 128)])
        cos, sin = _rope_tabs(pos)           # [64, 256] feature-major
        # row-major per-slot tables tiled 4x along free: [2 slot, 2 (cos,sin), 128, 256]
        rt = np.zeros((2, 2, 128, 256), np.float32)
        for slot_i in range(2):
            cr = cos[:, slot_i * 128:(slot_i + 1) * 128].T    # [128, 64]
            sr = sin[:, slot_i * 128:(slot_i + 1) * 128].T
            rt[slot_i, 0] = np.tile(cr, (1, 4))
            rt[slot_i, 1] = np.tile(sr, (1, 4))
        # masks [6, 128, 512]: 4 pair-tiles (units 0..7, [L|H0|L|H0]) then
        # 2 quad-tiles (units 8..15, H-stripe only)
        m = np.zeros((6, 128, 512), np.float32)
        for p_ in range(4):
            m[p_, :, 0:128] = _mask_for(sl, 2 * p_)
            m[p_, :, 256:384] = _mask_for(sl, 2 * p_ + 1)
            # H-stripe columns of pair units: chunks 0..7 are always visible
            m[p_, :, 128:256] = 1.0
            m[p_, :, 384:512] = 1.0
        for q_ in range(2):
            for u_ in range(4):
                m[4 + q_, :, u_ * 128:(u_ + 1) * 128] = _mask_for(sh, 8 + q_ * 4 + u_)
        im = {
            "x_rows": np.stack([x[sl * 128:(sl + 1) * 128],
                                x[sh * 128:(sh + 1) * 128]]),
            "rtab": rt,
            "masks": m.astype(BF),
            "biases": shared["biases"],
            "wqkvT": shared["wqkvT"], "woT": shared["woT"],
            "w1S": _wimg(w1[c * FSH:(c + 1) * FSH]),
            "w3S": _wimg(w3[c * FSH:(c + 1) * FSH]),
            "w2T": np.ascontiguousarray(
                w2[:, c * FSH:(c + 1) * FSH].T).astype(BF),
        }
        in_maps.append(im)
    return in_maps


def kernel(**inputs) -> np.ndarray:
    nc = _get_nc()
    in_maps = _prep_in_maps(inputs)
    res = run_bass_kernel_spmd(nc, in_maps, core_ids=list(range(NCORES)))
    out = np.empty((1, N, DIM), np.float32)
    for c in range(NCORES):
        o = res.results[c]["out"]            # [2, 128, DIM]
        out[0, c * 128:(c + 1) * 128] = o[0]
        out[0, (NCH - 1 - c) * 128:(NCH - c) * 128] = o[1]
    return out



# revision 30
# speedup vs baseline: 1.0466x; 1.0466x over previous
"""Fused pre-norm decoder layer (RMSNorm + GQA causal attention w/ RoPE +
RMSNorm + SwiGLU MLP) on 8 Trainium2 NeuronCores.

Sharding: sequence-parallel with folded stripe pairs — core c owns row stripes
{c, 15-c} (128 rows each) so causal attention work is balanced; the MLP is
tensor-parallel (w1/w3 column-split, w2 row-split). Cross-core comms:
AllGather of roped K^T + V (bf16), AllGather of the transposed normed hidden
states (bf16), and a 4-way chunked ReduceScatter of the MLP partial outputs
(bf16) overlapped with the w2 matmuls. Matmuls in bf16, residuals in fp32.

Self-contained: hardcodes the reference shapes
(B=1, N=2048, DIM=2048, HQ=16, HK=4, HD=128, F=8192).
"""
import numpy as np
import ml_dtypes

import concourse.bass as bass
import concourse.mybir as mybir
import concourse.tile as tile
from concourse import bacc
from concourse.bass_utils import run_bass_kernel_spmd
from concourse.masks import make_identity

F32 = mybir.dt.float32
BF16 = mybir.dt.bfloat16
AF = mybir.ActivationFunctionType
ALU = mybir.AluOpType
BF = ml_dtypes.bfloat16

DIM = 2048
HQ = 16            # query heads
HK = 4             # kv heads
HD = 128           # head dim
KV = HD * HK       # 512
N = 2048           # sequence length
FF = 4 * DIM       # 8192 mlp hidden
EPS = 1e-6
ROPE_BASE = 10000.0
SCALE = HD ** -0.5

NCORES = 8
RG = [list(range(NCORES))]
NCH = N // 128       # 16 sequence chunks
NIC = DIM // 128     # 16 feature chunks
FSH = FF // NCORES   # 1024 mlp hidden per core
FSC = FSH // 128     # 8 f-chunks per core
NEG = -1e30
DEBUG = False

# core c owns stripes (c, 15-c); local rows = [stripe_c | stripe_{15-c}]
# global s-chunk j lives on core own(j), slot slot(j):
def _owner(j):
    return (j, 0) if j < NCH // 2 else (NCH - 1 - j, 1)


def _build_kernel():
    nc = bacc.Bacc(None, target_bir_lowering=False)

    x_rows = nc.dram_tensor("x_rows", [2, 128, DIM], F32, kind="ExternalInput")
    rtab = nc.dram_tensor("rtab", [2, 2, 128, 256], F32, kind="ExternalInput")
    masks = nc.dram_tensor("masks", [6, 128, 512], BF16, kind="ExternalInput")
    biases = nc.dram_tensor("biases", [2, 3072], BF16, kind="ExternalInput")
    wqkvT = nc.dram_tensor("wqkvT", [DIM, 3072], BF16, kind="ExternalInput")
    woT = nc.dram_tensor("woT", [DIM, DIM], BF16, kind="ExternalInput")
    w1S = nc.dram_tensor("w1S", [FSC, 128, DIM], BF16, kind="ExternalInput")
    w3S = nc.dram_tensor("w3S", [FSC, 128, DIM], BF16, kind="ExternalInput")
    w2T = nc.dram_tensor("w2T", [FSH, DIM], BF16, kind="ExternalInput")
    out_ext = nc.dram_tensor("out", [2, 128, DIM], F32, kind="ExternalOutput")
    dbg = {}
    if DEBUG:
        for nm, shp, dt in [("dbg_qkv", [128, 2, 3072], BF16),
                            ("dbg_attn", [128, 2, DIM], BF16),
                            ("dbg_h", [128, 2, DIM], F32),
                            ("dbg_xn", [128, 2, DIM], BF16),
                            ("dbg_xnT", [128, NIC, 256], BF16)]:
            dbg[nm] = nc.dram_tensor(nm, shp, dt, kind="ExternalOutput")

    with tile.TileContext(nc) as tc:
        _body(nc, tc, x_rows, rtab, masks, biases,
              wqkvT, woT, w1S, w3S, w2T, out_ext, dbg)
    nc.compile()
    return nc


def _rope_psum(nc, rp, rtab_sb, pcur, sl, dst):
    """rope a [128, 512] psum tile (4 head-blocks) into dst [128, 512] bf16."""
    AFv = mybir.ActivationFunctionType
    pv = pcur.rearrange("p (h t) -> p h t", t=128)
    cosT = rtab_sb[:, sl, 0, :].rearrange("p (h t) -> p h t", t=64)
    sinT = rtab_sb[:, sl, 1, :].rearrange("p (h t) -> p h t", t=64)
    t1 = rp.tile([128, 4, 64], F32, name="t1", tag="t1")
    t2 = rp.tile([128, 4, 64], F32, name="t2", tag="t2")
    t3 = rp.tile([128, 4, 64], F32, name="t3", tag="t3")
    t4 = rp.tile([128, 4, 64], F32, name="t4", tag="t4")
    nc.vector.tensor_mul(t1[:], pv[:, :, 0:64], cosT)
    nc.vector.tensor_mul(t2[:], pv[:, :, 64:128], sinT)
    nc.vector.tensor_mul(t3[:], pv[:, :, 0:64], sinT)
    nc.vector.tensor_mul(t4[:], pv[:, :, 64:128], cosT)
    dstv = dst.rearrange("p (h t) -> p h t", t=128)
    nc.vector.tensor_sub(dstv[:, :, 0:64], t1[:], t2[:])
    nc.vector.tensor_add(dstv[:, :, 64:128], t3[:], t4[:])


def _rmsnorm_to(nc, pool, out_bf, x_sb, slot, eps_tile, scratch):
    """out_bf[:, slot, :] = rmsnorm(x_sb[:, slot, :]) cast bf16.
    scratch: any writable [128, DIM] f32 AP whose contents may be clobbered."""
    ssq = pool.tile([128, 1], F32, name="ssq", tag="ssq")
    nc.scalar.activation(scratch, x_sb[:, slot, :], AF.Square, accum_out=ssq[:])
    rms = pool.tile([128, 1], F32, name="rms", tag="rms")
    nc.scalar.activation(rms[:], ssq[:], AF.Sqrt, bias=eps_tile[:], scale=1.0 / DIM)
    rinv = pool.tile([128, 1], F32, name="rinv", tag="rinv")
    nc.vector.reciprocal(rinv[:], rms[:])
    nc.vector.tensor_scalar_mul(out_bf[:, slot, :], x_sb[:, slot, :], rinv[:])


def _transpose_2x16(nc, sb, ps, dst, src, ident, tag):
    """src [128, 2, 2048] bf16 row-major -> dst [128, 16, 256] bf16 (feature-major).
    dst[:, ic, s*128:(s+1)*128] = src[:, s, ic*128:(ic+1)*128].T
    """
    for s in range(2):
        for ic in range(NIC):
            tp = ps.tile([128, 128], BF16, name=f"tp_{tag}", tag=f"tp_{tag}")
            nc.tensor.transpose(tp[:], src[:, s, ic * 128:(ic + 1) * 128], ident[:])
            nc.vector.tensor_copy(dst[:, ic, s * 128:(s + 1) * 128], tp[:])


def _body(nc, tc, x_rows, rtab, masks, biases,
          wqkvT, woT, w1S, w3S, w2T, out_ext, dbg={}):
    import contextlib
    ctx = contextlib.ExitStack()
    with ctx:
        const = ctx.enter_context(tc.tile_pool(name="const", bufs=1))
        persist = ctx.enter_context(tc.tile_pool(name="persist", bufs=1))
        dram = ctx.enter_context(tc.tile_pool(name="dram", bufs=1, space="DRAM"))
        small = ctx.enter_context(tc.tile_pool(name="small", bufs=4))

        ident = const.tile([128, 128], BF16)
        make_identity(nc, ident)
        eps_tile = const.tile([128, 1], F32)
        nc.gpsimd.memset(eps_tile[:], EPS)
        ones_bf = const.tile([1, 512], BF16)
        nc.gpsimd.memset(ones_bf[:], 1.0)
        bias_sb = None  # allocated in attention pool below

        # DRAM comm buffers
        agk_in = dram.tile([HK * 128 * 256], BF16)
        agk_out = dram.tile([NCORES, HK * 128 * 256], BF16,
                            addr_space="Shared")
        agv_in = dram.tile([2 * 128 * KV], BF16)
        agv_out = dram.tile([NCORES, 2 * 128 * KV], BF16,
                            addr_space="Shared")
        agx_in = dram.tile([2, 2, 8 * 128 * 128], BF16)
        agx_outs = [[dram.tile([NCORES, 8 * 128 * 128], BF16,
                               addr_space="Shared", name=f"agxo{s_}{h_}")
                     for h_ in range(2)] for s_ in range(2)]
        rs_in = dram.tile([4, 512, DIM], BF16)
        rs_out = dram.tile([4, 64, DIM], BF16)

        # persistent SBUF
        h_sb = persist.tile([128, 2, DIM], F32)       # post-attention residual
        x2nT = persist.tile([128, NIC, 256], BF16)

        # attention-phase pool: closed before the MLP pool allocates
        att_ctx = contextlib.ExitStack()
        ph1 = att_ctx.enter_context(tc.tile_pool(name="ph1", bufs=1))
        qkv_ctx = contextlib.ExitStack()
        qkvp = qkv_ctx.enter_context(tc.tile_pool(name="qkvp", bufs=1))
        rtab_sb = qkvp.tile([128, 2, 2, 256], F32)
        nc.gpsimd.dma_start(rtab_sb[:], rtab.rearrange("s c p t -> p s c t"))
        x_sb = ph1.tile([128, 2, DIM], F32)           # own rows [slotL | slotH]
        for s in range(2):
            eng = nc.sync if s == 0 else nc.scalar
            eng.dma_start(x_sb[:, s, :], x_rows[s])
        # K/V projection weights fully resident so the K AllGather can launch
        # as early as possible (critical path of the whole kernel)
        wkv_ctx = contextlib.ExitStack()
        wkvres = wkv_ctx.enter_context(tc.tile_pool(name="wkvres", bufs=1))
        wkv_sb = wkvres.tile([128, NIC, 1024], BF16)
        _wkv_engs = [nc.gpsimd, nc.scalar, nc.sync, nc.scalar]
        for hf in range(4):
            _wkv_engs[hf].dma_start(
                wkv_sb[:, hf * 4:(hf + 1) * 4, :],
                wqkvT[:, 0:1024].rearrange(
                    "(i p) c -> p i c", p=128)[:, hf * 4:(hf + 1) * 4, :])
        # PE warmup: DMA-independent matmuls raise HAM to K=8/8 while the
        # prologue DMAs/norm run; result sunk to DRAM to stay live.
        warm_sink = dram.tile([128, 1], F32)
        with (
            tc.tile_pool(name="warmp", bufs=1, space="PSUM") as warmp,
            tc.tile_pool(name="warms", bufs=1) as warms,
        ):
            wrm = warms.tile([128, 512], BF16)
            nc.gpsimd.memset(wrm[:], 1.0)
            wps = warmp.tile([128, 512], F32)
            for wi in range(8):
                nc.tensor.matmul(wps[:], ident[:], wrm[:],
                                 start=True, stop=True)
            wsb = warms.tile([128, 1], F32)
            nc.vector.tensor_copy(wsb[:], wps[:, 0:1])
            nc.sync.dma_start(warm_sink[:], wsb[:])
        bias_sb = ph1.tile([1, 2, 3072], BF16)
        for i in range(2):
            nc.sync.dma_start(bias_sb[0:1, i, :], biases[i:i + 1, :])
        mask_sb = ph1.tile([128, 6, 512], BF16)
        nc.sync.dma_start(mask_sb[:], masks.rearrange("k p q -> p k q"))
        xn = qkvp.tile([128, 2, DIM], BF16)
        for s in range(2):
            _rmsnorm_to(nc, small, xn, x_sb, s, eps_tile, h_sb[:, s, :])
        xnT = qkvp.tile([128, NIC, 256], BF16)

        # ===== phase 2a: K/V projections + rope-k + early AllGathers =====
        # qkv_rows[:, slot, 0:2048]=roped q, [2048:2560]=roped k, [2560:3072]=v
        # wqkvT col order: [k(512) | v(512) | q(2048)]
        # xn transposes are interleaved into the K/V matmul loop (transpose for
        # ic runs ~4 iterations ahead of its consuming matmuls).
        qkv_rows = qkvp.tile([128, 2, 3072], BF16)
        kT_own = qkvp.tile([128, HK, 256], BF16)
        q_roped = ph1.tile([128, HQ, 256], BF16)

        def _xn_transpose(ps1, ic):
            for s in range(2):
                tp = ps1.tile([128, 128], BF16, name="tp_xn", tag="tp_xn")
                nc.tensor.transpose(tp[:], xn[:, s, ic * 128:(ic + 1) * 128],
                                    ident[:])
                nc.vector.tensor_copy(xnT[:, ic, s * 128:(s + 1) * 128], tp[:])

        with (
            tc.tile_pool(name="pkv", bufs=1, space="PSUM") as pkv,
            tc.tile_pool(name="rp", bufs=2) as rp,
        ):
            ps = [pkv.tile([128, 512], F32, name=f"pkv{u}", tag=f"pkv{u}")
                  for u in range(4)]          # (oc, slot): oc0=k, oc1=v
            with tc.tile_pool(name="tp1", bufs=3, space="PSUM") as ps1:
                for ic in range(4):
                    _xn_transpose(ps1, ic)
                for ic in range(NIC):
                    if ic + 4 < NIC:
                        _xn_transpose(ps1, ic + 4)
                    for oi in range(2):
                        for sl in range(2):
                            nc.tensor.matmul(
                                ps[oi * 2 + sl][:],
                                xnT[:, ic, sl * 128:(sl + 1) * 128],
                                wkv_sb[:, ic, oi * 512:(oi + 1) * 512],
                                start=(ic == 0), stop=False)
            for oi in range(2):
                for sl in range(2):
                    nc.tensor.matmul(
                        ps[oi * 2 + sl][:], ones_bf[:, 0:128],
                        bias_sb[:, 0, oi * 512:(oi + 1) * 512],
                        start=False, stop=True)
            # k: rope then transpose + ship + AG (K first: attention scores
            # gate on it); v: plain copy, ship + AG second.
            for sl in range(2):
                _rope_psum(nc, rp, rtab_sb, ps[0 + sl],
                           sl, qkv_rows[:, sl, 2048:2560])
            with tc.tile_pool(name="tpk", bufs=2, space="PSUM") as tpk:
                for sl in range(2):
                    for kh in range(HK):
                        tp = tpk.tile([128, 128], BF16, name="tp_k",
                                      tag="tp_k")
                        nc.tensor.transpose(
                            tp[:],
                            qkv_rows[:, sl,
                                     2048 + kh * 128:2048 + (kh + 1) * 128],
                            ident[:])
                        nc.vector.tensor_copy(
                            kT_own[:, kh, sl * 128:(sl + 1) * 128], tp[:])
            nc.sync.dma_start(
                agk_in.rearrange("(k d n) -> d k n", k=HK, d=128),
                kT_own[:])
            nc.gpsimd.collective_compute(
                "AllGather", ALU.bypass, replica_groups=RG,
                ins=[agk_in.opt()], outs=[agk_out.opt()])
            for sl in range(2):
                nc.vector.tensor_copy(qkv_rows[:, sl, 2560:3072],
                                      ps[2 + sl][:])
            nc.sync.dma_start(
                agv_in.rearrange("(t2 t k) -> t t2 k", t2=2, t=128),
                qkv_rows[:, :, 2560:3072])
            nc.gpsimd.collective_compute(
                "AllGather", ALU.bypass, replica_groups=RG,
                ins=[agv_in.opt()], outs=[agv_out.opt()])

        wkv_ctx.close()
        # ===== phase 2b: Q projections + rope + transposes (overlap AGs) ====
        with (
            tc.tile_pool(name="wq", bufs=2) as wqp,
            tc.tile_pool(name="pq", bufs=1, space="PSUM") as pq,
            tc.tile_pool(name="rp2", bufs=2) as rp2,
        ):
            psq = [pq.tile([128, 512], F32, name=f"pq{u}", tag=f"pq{u}")
                   for u in range(8)]         # (oc, slot)
            for ic in range(NIC):
                w_t = wqp.tile([128, 2048], BF16, name="wq_t", tag="wqt")
                nc.sync.dma_start(
                    w_t[:], wqkvT[ic * 128:(ic + 1) * 128, 1024:3072])
                for oi in range(4):
                    for sl in range(2):
                        nc.tensor.matmul(
                            psq[oi * 2 + sl][:],
                            xnT[:, ic, sl * 128:(sl + 1) * 128],
                            w_t[:, oi * 512:(oi + 1) * 512],
                            start=(ic == 0), stop=False)
            for oi in range(4):
                for sl in range(2):
                    nc.tensor.matmul(
                        psq[oi * 2 + sl][:], ones_bf[:, 0:128],
                        bias_sb[:, 0, 1024 + oi * 512:1024 + (oi + 1) * 512],
                        start=False, stop=True)
            for oi in range(4):
                for sl in range(2):
                    _rope_psum(nc, rp2, rtab_sb, psq[oi * 2 + sl],
                               sl, qkv_rows[:, sl, oi * 512:(oi + 1) * 512])
        # transposes: q -> q_roped [d, h, n] (head-ascending: scores gate on
        # low heads first)
        with tc.tile_pool(name="tpq", bufs=3, space="PSUM") as tpq:
            for h in range(HQ):
                for sl in range(2):
                    tp = tpq.tile([128, 128], BF16, name="tp_q", tag="tp_q")
                    nc.tensor.matmul(tp[:], qkv_rows[:, sl, h * 128:(h + 1) * 128],
                                     ident[:], is_transpose=True)
                    nc.vector.tensor_copy(q_roped[:, h, sl * 128:(sl + 1) * 128], tp[:])
        if dbg:
            nc.sync.dma_start(dbg["dbg_qkv"][:], qkv_rows[:])
            nc.sync.dma_start(dbg["dbg_xn"][:], xn[:])
            nc.sync.dma_start(dbg["dbg_xnT"][:], xnT[:])
        qkv_ctx.close()

        # ============ phase 4: gather K/V into SBUF (rank-major layouts) ============
        # kT_full[:, kh, r, slot*128+t] = rank r's K slot cols; unit code indexes
        # via _owner(j) -> (r, slot).  K gathers first (scores gate on them),
        # spread over 3 queues.
        kT_full = ph1.tile([128, HK, NCORES, 256], BF16)
        v_aug = ph1.tile([128, NCORES, 2, HK, 132], BF16)
        kengs = [nc.sync, nc.gpsimd, nc.scalar]
        for r in range(NCORES):
            kengs[r % 3].dma_start(
                kT_full[:, :, r, :],
                agk_out[r].rearrange("(k d n) -> d k n", k=HK, d=128))
        nc.gpsimd.memset(v_aug[:, :, :, :, 128:129], 1.0)
        for r in range(NCORES):
            vsrc = agv_out[r].rearrange(
                "(t2 t k d) -> t t2 k d", t2=2, t=128, k=HK)
            for sl2 in range(2):
                kengs[r % 3].dma_start(v_aug[:, r, sl2, :, 0:128],
                                       vsrc[:, sl2, :, :])

        # ============ phase 5: attention ============
        # units 0..7 (pairs of s-chunks vs both stripes) packed 2-per-psum-bank;
        # units 8..15 (H-stripe only) packed 4-per-bank. One stt+exp per bank.
        # Score phases are hoisted AV_DEPTH heads ahead of AV phases so the
        # PE keeps scoring while the V AllGather is still in flight, and the
        # vector/scalar mask+exp chain pipelines ahead of the AV matmuls.
        attn = ph1.tile([128, 2, DIM], BF16)     # row-major attn out (normalized)
        attnT = ph1.tile([128, NIC, 256], BF16)
        AV_DEPTH = 4
        with (
            tc.tile_pool(name="ps_sc", bufs=3, space="PSUM") as ps_sc,
            tc.tile_pool(name="ps_av", bufs=2, space="PSUM") as ps_av,
            tc.tile_pool(name="tp_at", bufs=1, space="PSUM") as tp_at,
            tc.tile_pool(name="att_sb", bufs=6 * (AV_DEPTH + 1)) as att_sbp,
            tc.tile_pool(name="att_tmp", bufs=3) as att_tmp,
        ):
            def scores_phase(h):
                kh = h % HK
                att_tiles = []
                for pair in range(4):            # units 2*pair, 2*pair+1
                    sc = ps_sc.tile([128, 512], F32, name="sc", tag="sc")
                    for u in range(2):
                        k = pair * 2 + u
                        rk, sk = _owner(k)
                        nc.tensor.matmul(
                            sc[:, u * 256:(u + 1) * 256],
                            kT_full[:, kh, rk, sk * 128:(sk + 1) * 128],
                            q_roped[:, h, :], start=True, stop=True)
                    tmp = att_tmp.tile([128, 512], BF16, name="mtmp", tag="mtmp")
                    nc.scalar.activation(tmp[:], sc[:], AF.Exp, scale=SCALE)
                    att = att_sbp.tile([128, 512], BF16, name="attP", tag="attP")
                    nc.vector.tensor_mul(att[:], tmp[:], mask_sb[:, pair, :])
                    att_tiles.append(att)
                for quad in range(2):            # units 8..11, 12..15 (H only)
                    sc = ps_sc.tile([128, 512], F32, name="sc", tag="sc")
                    for u in range(4):
                        k = 8 + quad * 4 + u
                        rk, sk = _owner(k)
                        nc.tensor.matmul(
                            sc[:, u * 128:(u + 1) * 128],
                            kT_full[:, kh, rk, sk * 128:(sk + 1) * 128],
                            q_roped[:, h, 128:256], start=True, stop=True)
                    tmp = att_tmp.tile([128, 512], BF16, name="mtmp", tag="mtmp")
                    nc.scalar.activation(tmp[:], sc[:], AF.Exp, scale=SCALE)
                    att = att_sbp.tile([128, 512], BF16, name="attP", tag="attP")
                    nc.vector.tensor_mul(att[:], tmp[:], mask_sb[:, 4 + quad, :])
                    att_tiles.append(att)
                return att_tiles

            def av_phase(h, att_tiles):
                kh = h % HK
                av = [ps_av.tile([128, 132], F32, name=f"av{s}", tag=f"av{s}")
                      for s in range(2)]
                for pair in range(4):
                    att = att_tiles[pair]
                    for u in range(2):
                        k = pair * 2 + u
                        rk, sk = _owner(k)
                        vap = v_aug[:, rk, sk, kh, 0:129]
                        nc.tensor.matmul(
                            av[0][:, 0:129], att[:, u * 256:u * 256 + 128],
                            vap, start=(k == 0), stop=(k == 7))
                        nc.tensor.matmul(
                            av[1][:, 0:129], att[:, u * 256 + 128:u * 256 + 256],
                            vap, start=(k == 0), stop=False)
                for quad in range(2):
                    att = att_tiles[4 + quad]
                    for u in range(4):
                        k = 8 + quad * 4 + u
                        rk, sk = _owner(k)
                        nc.tensor.matmul(
                            av[1][:, 0:129], att[:, u * 128:(u + 1) * 128],
                            v_aug[:, rk, sk, kh, 0:129],
                            start=False, stop=(k == NCH - 1))
                # normalize by denominator (col 128), then transpose this
                # head's column into attnT right away
                for s in range(2):
                    rd = small.tile([128, 1], F32, name="rd", tag="rd")
                    nc.vector.reciprocal(rd[:], av[s][:, 128:129])
                    nc.vector.tensor_scalar_mul(
                        attn[:, s, h * 128:(h + 1) * 128], av[s][:, 0:128], rd[:])
                for s in range(2):
                    tp = tp_at.tile([128, 128], BF16, name="tp_a", tag="tp_a")
                    nc.tensor.transpose(
                        tp[:], attn[:, s, h * 128:(h + 1) * 128], ident[:])
                    nc.vector.tensor_copy(
                        attnT[:, h, s * 128:(s + 1) * 128], tp[:])

            pend = []
            for h in range(HQ):
                pend.append((h, scores_phase(h)))
                if len(pend) > AV_DEPTH:
                    av_phase(*pend.pop(0))
            for item in pend:
                av_phase(*item)

        if dbg:
            nc.sync.dma_start(dbg["dbg_attn"][:], attn[:])

        # ============ phase 6: output projection + residual ============
        with (
            tc.tile_pool(name="wo", bufs=6) as wop,
            tc.tile_pool(name="po", bufs=1, space="PSUM") as po,
        ):
            pso = [po.tile([128, 512], F32, name=f"pso{i}", tag=f"pso{i}")
                   for i in range(8)]      # (slot, oc4)
            for ic in range(NIC):
                wo_t = wop.tile([128, DIM], BF16, name="wo_t", tag="wo")
                eng = nc.sync if ic % 2 == 0 else nc.gpsimd
                eng.dma_start(wo_t[:], woT[ic * 128:(ic + 1) * 128, :])
                for s in range(2):
                    for oc in range(4):
                        nc.tensor.matmul(
                            pso[s * 4 + oc][:],
                            attnT[:, ic, s * 128:(s + 1) * 128],
                            wo_t[:, oc * 512:(oc + 1) * 512],
                            start=(ic == 0), stop=False)
            for s in range(2):
                for oc in range(4):
                    nc.tensor.matmul(
                        pso[s * 4 + oc][:], ones_bf[:, 0:128],
                        bias_sb[:, 1, oc * 512:(oc + 1) * 512],
                        start=False, stop=True)
            for s in range(2):
                for oc in range(4):
                    nc.vector.tensor_add(
                        h_sb[:, s, oc * 512:(oc + 1) * 512],
                        pso[s * 4 + oc][:], x_sb[:, s, oc * 512:(oc + 1) * 512])

        if dbg:
            nc.sync.dma_start(dbg["dbg_h"].rearrange("p s d -> p s d"), h_sb[:])
        # ============ phase 7: norm2 + transpose + per-slot group AG ============
        # AG for slot L ships as soon as its 16 transposes land so the MLP's
        # L-half can start while slot H is still being normed/gathered.
        x2n = ph1.tile([128, 2, DIM], BF16)
        with tc.tile_pool(name="tp3", bufs=3, space="PSUM") as ps3:
            for s in range(2):
                _rmsnorm_to(nc, small, x2n, h_sb, s, eps_tile, x_sb[:, s, :])
                for hf in range(2):
                    for ic in range(hf * 8, hf * 8 + 8):
                        tp = ps3.tile([128, 128], BF16, name="tp_x2", tag="tp_x2")
                        nc.tensor.transpose(
                            tp[:], x2n[:, s, ic * 128:(ic + 1) * 128], ident[:])
                        nc.vector.tensor_copy(
                            x2nT[:, ic, s * 128:(s + 1) * 128], tp[:])
                    eng = nc.sync if hf == 0 else nc.scalar
                    eng.dma_start(
                        agx_in[s, hf].rearrange("(i p t) -> p i t", i=8, p=128),
                        x2nT[:, hf * 8:(hf + 1) * 8, s * 128:(s + 1) * 128])
                    nc.gpsimd.collective_compute(
                        "AllGather", ALU.bypass, replica_groups=RG,
                        ins=[agx_in[s, hf].opt()],
                        outs=[agx_outs[s][hf].opt()])

        # ============ phase 8: MLP (TP, FF/8) with chunked RS ============
        # g-chunks 0,1 read only slot-L columns (nbase < 128) so they gate on
        # the slot-L AllGather alone; slot-H arrives while g=0,1 compute.
        att_ctx.close()
        mlpw = ctx.enter_context(tc.tile_pool(name="mlpw", bufs=1, side="right"))
        w2_sb = mlpw.tile([128, FSC, DIM], BF16)
        for hf in range(2):
            eng = nc.gpsimd if hf == 0 else nc.scalar
            eng.dma_start(
                w2_sb[:, hf * 4:(hf + 1) * 4, :],
                w2T.rearrange("(f p) o -> p f o", p=128)[:, hf * 4:(hf + 1) * 4, :])
        mlp = ctx.enter_context(tc.tile_pool(name="mlp", bufs=1, side="right"))
        x2nT_full = mlp.tile([128, NIC, N], BF16)
        for s in range(2):
            for hf in range(2):
                for r in range(NCORES):
                    kengs[r % 3].dma_start(
                        x2nT_full[:, hf * 8:(hf + 1) * 8,
                                  r * 256 + s * 128:r * 256 + (s + 1) * 128],
                        agx_outs[s][hf][r].rearrange(
                            "(i p t) -> p i t", i=8, p=128))

        w13p = mlpw
        with (
            tc.tile_pool(name="ps_y", bufs=2, space="PSUM") as ps_y,
            tc.tile_pool(name="h2p", bufs=2) as h2p,
            tc.tile_pool(name="ps_w2", bufs=2, space="PSUM") as ps_w2,
            tc.tile_pool(name="rs_sb", bufs=3) as rs_sbp,
        ):
            for g in range(4):             # n-super-chunk = P2 block g
                # rhs columns: P1 cols {r*256 + (g//2)*128 + (g%2)*64 .. +64}
                nbase = (g // 2) * 128 + (g % 2) * 64
                h2T = h2p.tile([128, FSC, 512], BF16, name="h2T", tag="h2T")
                for f in range(FSC):
                    w1_t = w13p.tile([128, NIC, 128], BF16, name="w1_t", tag="w1", bufs=3)
                    nc.sync.dma_start(
                        w1_t.rearrange("p i f -> p (i f)"), w1S[f])
                    w3_t = w13p.tile([128, NIC, 128], BF16, name="w3_t", tag="w3", bufs=3)
                    nc.gpsimd.dma_start(
                        w3_t.rearrange("p i f -> p (i f)"), w3S[f])
                    y1 = ps_y.tile([128, 512], F32, name="y1", tag="y1")
                    y3 = ps_y.tile([128, 512], F32, name="y3", tag="y3")
                    for ic in range(NIC):
                        rhs = x2nT_full[:, ic, :].rearrange(
                            "p (r t) -> p r t", t=256)[:, :, nbase:nbase + 64]
                        nc.tensor.matmul(y1[:], w1_t[:, ic, :], rhs,
                                         start=(ic == 0), stop=(ic == NIC - 1))
                        nc.tensor.matmul(y3[:], w3_t[:, ic, :], rhs,
                                         start=(ic == 0), stop=(ic == NIC - 1))
                    sg = rs_sbp.tile([128, 512], BF16, name="sg", tag="sg")
                    nc.scalar.activation(sg[:], y1[:], AF.Sigmoid)
                    sil = rs_sbp.tile([128, 512], F32, name="sil", tag="sil")
                    nc.vector.scalar_tensor_tensor(
                        sil[:], y1[:], 1.0, sg[:], op0=ALU.mult, op1=ALU.mult)
                    nc.vector.tensor_mul(h2T[:, f, :], sil[:], y3[:])
                # w2: out partial rows for P2 block g
                for q in range(4):         # 128-row slices within block
                    for oc in range(4):
                        pw = ps_w2.tile([128, 512], F32, name="pw", tag="pw")
                        for f in range(FSC):
                            nc.tensor.matmul(
                                pw[:], h2T[:, f, q * 128:(q + 1) * 128],
                                w2_sb[:, f, oc * 512:(oc + 1) * 512],
                                start=(f == 0), stop=(f == FSC - 1))
                        ob = rs_sbp.tile([128, 512], BF16, name="ob", tag="ob")
                        if (q * 4 + oc) % 2 == 0:
                            nc.vector.tensor_copy(ob[:], pw[:])
                        else:
                            nc.scalar.copy(ob[:], pw[:])
                        nc.sync.dma_start(
                            rs_in[g, q * 128:(q + 1) * 128,
                                  oc * 512:(oc + 1) * 512], ob[:])
                nc.gpsimd.collective_compute(
                    "ReduceScatter", ALU.add, replica_groups=RG,
                    ins=[rs_in[g].opt()], outs=[rs_out[g].opt()])

        # ============ phase 9: final residual + output (per RS chunk) ============
        rs_res = mlp.tile([128, 2, DIM], BF16)
        out_sb = mlp.tile([128, 2, DIM], F32)
        for g in range(4):
            s, half = g // 2, g % 2
            pr = slice(half * 64, (half + 1) * 64)
            eng = nc.sync if g % 2 == 0 else nc.gpsimd
            eng.dma_start(rs_res[pr, s, :], rs_out[g])
            nc.vector.tensor_add(out_sb[pr, s, :], rs_res[pr, s, :], h_sb[pr, s, :])
            eng.dma_start(out_ext[s, pr, :], out_sb[pr, s, :])


# ============================ host side ============================

def _perm(nheads):
    p = []
    for h in range(nheads):
        base = h * HD
        p.extend(range(base, base + HD, 2))
        p.extend(range(base + 1, base + HD, 2))
    return np.array(p)


def _rope_tabs(pos):
    inv = 1.0 / (ROPE_BASE ** (np.arange(0, HD, 2, dtype=np.float32) / HD))
    f = np.outer(pos.astype(np.float32), inv)        # [n, 64]
    return np.cos(f).T.astype(np.float32), np.sin(f).T.astype(np.float32)


def _mask_for(stripe, j):
    """multiplicative mask [128 s, 128 q] for s-chunk j vs q-stripe `stripe`"""
    if j < stripe:
        return np.ones((128, 128), np.float32)
    if j > stripe:
        return np.zeros((128, 128), np.float32)
    i = np.arange(128)
    return np.where(i[:, None] <= i[None, :], 1.0, 0.0).astype(np.float32)


def _wimg(wshard):
    """[1024, 2048] w-shard -> SBUF images [8 f-chunks, 128 part(i%128), 16*128]
    img[f][p, ic*128+t] = w.T[ic*128+p, f*128+t]"""
    wT = wshard.T                      # [2048 i, 1024 f]
    img = wT.reshape(NIC, 128, FSC, 128).transpose(2, 1, 0, 3).reshape(FSC, 128, DIM)
    return np.ascontiguousarray(img).astype(BF)


_CACHED_NC = None


def _get_nc():
    global _CACHED_NC
    if _CACHED_NC is None:
        _CACHED_NC = _build_kernel()
    return _CACHED_NC


def _prep_in_maps(inputs):
    f32 = lambda a: np.ascontiguousarray(np.asarray(a), dtype=np.float32)
    x = f32(inputs["x"])[0]                  # [N, DIM]
    g_attn, g_mlp = f32(inputs["g_attn"]), f32(inputs["g_mlp"])
    pq, pk = _perm(HQ), _perm(HK)
    wq = f32(inputs["wq"])[pq] * g_attn[None, :]
    wk = f32(inputs["wk"])[pk] * g_attn[None, :]
    wv = f32(inputs["wv"]) * g_attn[None, :]
    wo = f32(inputs["wo"])
    w1 = f32(inputs["w1"]) * g_mlp[None, :]
    w3 = f32(inputs["w3"]) * g_mlp[None, :]
    w2 = f32(inputs["w2"])
    biases = np.zeros((2, 3072), np.float32)
    biases[0, 0:KV] = f32(inputs["bk"])[pk]
    biases[0, KV:2 * KV] = f32(inputs["bv"])
    biases[0, 2 * KV:] = f32(inputs["bq"])[pq]
    biases[1, 0:DIM] = f32(inputs["bo"])

    wqkv = np.concatenate([wk, wv, wq], 0)         # [3072, 2048] (k|v|q)
    shared = {
        "wqkvT": np.ascontiguousarray(wqkv.T).astype(BF),
        "woT": np.ascontiguousarray(wo.T).astype(BF),
        "biases": biases.astype(BF),
    }
    in_maps = []
    for c in range(NCORES):
        sl, sh = c, NCH - 1 - c
        pos = np.concatenate([np.arange(sl * 128, (sl + 1) * 128),
                              np.arange(sh * 128, (sh + 1) * 128)])
        cos, sin = _rope_tabs(pos)           # [64, 256] feature-major
        # row-major per-slot tables tiled 4x along free: [2 slot, 2 (cos,sin), 128, 256]
        rt = np.zeros((2, 2, 128, 256), np.float32)
        for slot_i in range(2):
            cr = cos[:, slot_i * 128:(slot_i + 1) * 128].T    # [128, 64]
            sr = sin[:, slot_i * 128:(slot_i + 1) * 128].T
            rt[slot_i, 0] = np.tile(cr, (1, 4))
            rt[slot_i, 1] = np.tile(sr, (1, 4))
        # masks [6, 128, 512]: 4 pair-tiles (units 0..7, [L|H0|L|H0]) then
        # 2 quad-tiles (units 8..15, H-stripe only)
        m = np.zeros((6, 128, 512), np.float32)
        for p_ in range(4):
            m[p_, :, 0:128] = _mask_for(sl, 2 * p_)
            m[p_, :, 256:384] = _mask_for(sl, 2 * p_ + 1)
            # H-stripe columns of pair units: chunks 0..7 are always visible
            m[p_, :, 128:256] = 1.0
            m[p_, :, 384:512] = 1.0
        for q_ in range(2):
            for u_ in range(4):
                m[4 + q_, :, u_ * 128:(u_ + 1) * 128] = _mask_for(sh, 8 + q_ * 4 + u_)
        im = {
            "x_rows": np.stack([x[sl * 128:(sl + 1) * 128],
                                x[sh * 128:(sh + 1) * 128]]),
            "rtab": rt,
            "masks": m.astype(BF),
            "biases": shared["biases"],
            "wqkvT": shared["wqkvT"], "woT": shared["woT"],
            "w1S": _wimg(w1[c * FSH:(c + 1) * FSH]),
            "w3S": _wimg(w3[c * FSH:(c + 1) * FSH]),
            "w2T": np.ascontiguousarray(
                w2[:, c * FSH:(c + 1) * FSH].T).astype(BF),
        }
        in_maps.append(im)
    return in_maps


def kernel(**inputs) -> np.ndarray:
    nc = _get_nc()
    in_maps = _prep_in_maps(inputs)
    res = run_bass_kernel_spmd(nc, in_maps, core_ids=list(range(NCORES)))
    out = np.empty((1, N, DIM), np.float32)
    for c in range(NCORES):
        o = res.results[c]["out"]            # [2, 128, DIM]
        out[0, c * 128:(c + 1) * 128] = o[0]
        out[0, (NCH - 1 - c) * 128:(NCH - c) * 128] = o[1]
    return out



# revision 33
# speedup vs baseline: 1.0864x; 1.0381x over previous
"""Fused pre-norm decoder layer (RMSNorm + GQA causal attention w/ RoPE +
RMSNorm + SwiGLU MLP) on 8 Trainium2 NeuronCores.

Sharding: sequence-parallel with folded stripe pairs — core c owns row stripes
{c, 15-c} (128 rows each) so causal attention work is balanced; the MLP is
tensor-parallel (w1/w3 column-split, w2 row-split). Cross-core comms:
AllGather of roped K^T + V (bf16), AllGather of the transposed normed hidden
states (bf16), and a 4-way chunked ReduceScatter of the MLP partial outputs
(bf16) overlapped with the w2 matmuls. Matmuls in bf16, residuals in fp32.

Self-contained: hardcodes the reference shapes
(B=1, N=2048, DIM=2048, HQ=16, HK=4, HD=128, F=8192).
"""
import numpy as np
import ml_dtypes

import concourse.bass as bass
import concourse.mybir as mybir
import concourse.tile as tile
from concourse import bacc
from concourse.bass_utils import run_bass_kernel_spmd
from concourse.masks import make_identity

F32 = mybir.dt.float32
BF16 = mybir.dt.bfloat16
AF = mybir.ActivationFunctionType
ALU = mybir.AluOpType
BF = ml_dtypes.bfloat16

DIM = 2048
HQ = 16            # query heads
HK = 4             # kv heads
HD = 128           # head dim
KV = HD * HK       # 512
N = 2048           # sequence length
FF = 4 * DIM       # 8192 mlp hidden
EPS = 1e-6
ROPE_BASE = 10000.0
SCALE = HD ** -0.5

NCORES = 8
RG = [list(range(NCORES))]
NCH = N // 128       # 16 sequence chunks
NIC = DIM // 128     # 16 feature chunks
FSH = FF // NCORES   # 1024 mlp hidden per core
FSC = FSH // 128     # 8 f-chunks per core
NEG = -1e30
DEBUG = False

# core c owns stripes (c, 15-c); local rows = [stripe_c | stripe_{15-c}]
# global s-chunk j lives on core own(j), slot slot(j):
def _owner(j):
    return (j, 0) if j < NCH // 2 else (NCH - 1 - j, 1)


def _build_kernel():
    nc = bacc.Bacc(None, target_bir_lowering=False)

    x_rows = nc.dram_tensor("x_rows", [2, 128, DIM], F32, kind="ExternalInput")
    rtab = nc.dram_tensor("rtab", [2, 2, 128, 256], F32, kind="ExternalInput")
    masks = nc.dram_tensor("masks", [6, 128, 512], BF16, kind="ExternalInput")
    biases = nc.dram_tensor("biases", [2, 3072], BF16, kind="ExternalInput")
    wqkvT = nc.dram_tensor("wqkvT", [DIM, 3072], BF16, kind="ExternalInput")
    woT = nc.dram_tensor("woT", [DIM, DIM], BF16, kind="ExternalInput")
    w1S = nc.dram_tensor("w1S", [FSC, 128, DIM], BF16, kind="ExternalInput")
    w3S = nc.dram_tensor("w3S", [FSC, 128, DIM], BF16, kind="ExternalInput")
    w2T = nc.dram_tensor("w2T", [FSH, DIM], BF16, kind="ExternalInput")
    out_ext = nc.dram_tensor("out", [2, 128, DIM], F32, kind="ExternalOutput")
    dbg = {}
    if DEBUG:
        for nm, shp, dt in [("dbg_qkv", [128, 2, 3072], BF16),
                            ("dbg_attn", [128, 2, DIM], BF16),
                            ("dbg_h", [128, 2, DIM], F32),
                            ("dbg_xn", [128, 2, DIM], BF16),
                            ("dbg_xnT", [128, NIC, 256], BF16)]:
            dbg[nm] = nc.dram_tensor(nm, shp, dt, kind="ExternalOutput")

    with tile.TileContext(nc) as tc:
        _body(nc, tc, x_rows, rtab, masks, biases,
              wqkvT, woT, w1S, w3S, w2T, out_ext, dbg)
    nc.compile()
    return nc


def _rope_psum(nc, rp, rtab_sb, pcur, sl, dst):
    """rope a [128, 512] psum tile (4 head-blocks) into dst [128, 512] bf16."""
    AFv = mybir.ActivationFunctionType
    pv = pcur.rearrange("p (h t) -> p h t", t=128)
    cosT = rtab_sb[:, sl, 0, :].rearrange("p (h t) -> p h t", t=64)
    sinT = rtab_sb[:, sl, 1, :].rearrange("p (h t) -> p h t", t=64)
    t1 = rp.tile([128, 4, 64], F32, name="t1", tag="t1")
    t2 = rp.tile([128, 4, 64], F32, name="t2", tag="t2")
    t3 = rp.tile([128, 4, 64], F32, name="t3", tag="t3")
    t4 = rp.tile([128, 4, 64], F32, name="t4", tag="t4")
    nc.vector.tensor_mul(t1[:], pv[:, :, 0:64], cosT)
    nc.vector.tensor_mul(t2[:], pv[:, :, 64:128], sinT)
    nc.vector.tensor_mul(t3[:], pv[:, :, 0:64], sinT)
    nc.vector.tensor_mul(t4[:], pv[:, :, 64:128], cosT)
    dstv = dst.rearrange("p (h t) -> p h t", t=128)
    nc.vector.tensor_sub(dstv[:, :, 0:64], t1[:], t2[:])
    nc.vector.tensor_add(dstv[:, :, 64:128], t3[:], t4[:])


def _rmsnorm_to(nc, pool, out_bf, x_sb, slot, eps_tile, scratch):
    """out_bf[:, slot, :] = rmsnorm(x_sb[:, slot, :]) cast bf16.
    scratch: any writable [128, DIM] f32 AP whose contents may be clobbered."""
    ssq = pool.tile([128, 1], F32, name="ssq", tag="ssq")
    nc.scalar.activation(scratch, x_sb[:, slot, :], AF.Square, accum_out=ssq[:])
    rms = pool.tile([128, 1], F32, name="rms", tag="rms")
    nc.scalar.activation(rms[:], ssq[:], AF.Sqrt, bias=eps_tile[:], scale=1.0 / DIM)
    rinv = pool.tile([128, 1], F32, name="rinv", tag="rinv")
    nc.vector.reciprocal(rinv[:], rms[:])
    nc.vector.tensor_scalar_mul(out_bf[:, slot, :], x_sb[:, slot, :], rinv[:])


def _transpose_2x16(nc, sb, ps, dst, src, ident, tag):
    """src [128, 2, 2048] bf16 row-major -> dst [128, 16, 256] bf16 (feature-major).
    dst[:, ic, s*128:(s+1)*128] = src[:, s, ic*128:(ic+1)*128].T
    """
    for s in range(2):
        for ic in range(NIC):
            tp = ps.tile([128, 128], BF16, name=f"tp_{tag}", tag=f"tp_{tag}")
            nc.tensor.transpose(tp[:], src[:, s, ic * 128:(ic + 1) * 128], ident[:])
            nc.vector.tensor_copy(dst[:, ic, s * 128:(s + 1) * 128], tp[:])


def _body(nc, tc, x_rows, rtab, masks, biases,
          wqkvT, woT, w1S, w3S, w2T, out_ext, dbg={}):
    import contextlib
    ctx = contextlib.ExitStack()
    with ctx:
        const = ctx.enter_context(tc.tile_pool(name="const", bufs=1))
        persist = ctx.enter_context(tc.tile_pool(name="persist", bufs=1))
        dram = ctx.enter_context(tc.tile_pool(name="dram", bufs=1, space="DRAM"))
        small = ctx.enter_context(tc.tile_pool(name="small", bufs=4))

        ident = const.tile([128, 128], BF16)
        make_identity(nc, ident)
        eps_tile = const.tile([128, 1], F32)
        nc.gpsimd.memset(eps_tile[:], EPS)
        ones_bf = const.tile([1, 512], BF16)
        nc.gpsimd.memset(ones_bf[:], 1.0)
        bias_sb = None  # allocated in attention pool below

        # DRAM comm buffers
        agk_in = dram.tile([HK * 128 * 256], BF16)
        agk_out = dram.tile([NCORES, HK * 128 * 256], BF16,
                            addr_space="Shared")
        agv_in = dram.tile([2 * 128 * KV], BF16)
        agv_out = dram.tile([NCORES, 2 * 128 * KV], BF16,
                            addr_space="Shared")
        agx_in = dram.tile([2, NIC * 128 * 128], BF16)
        agx_outs = [dram.tile([NCORES, NIC * 128 * 128], BF16,
                              addr_space="Shared", name=f"agxo{s_}")
                    for s_ in range(2)]
        rs_in = dram.tile([4, 512, DIM], BF16)
        rs_out = dram.tile([4, 64, DIM], BF16)

        # persistent SBUF
        h_sb = persist.tile([128, 2, DIM], F32)       # post-attention residual
        x2nT = persist.tile([128, NIC, 256], BF16)

        # attention-phase pool: closed before the MLP pool allocates
        att_ctx = contextlib.ExitStack()
        ph1 = att_ctx.enter_context(tc.tile_pool(name="ph1", bufs=1))
        qkv_ctx = contextlib.ExitStack()
        qkvp = qkv_ctx.enter_context(tc.tile_pool(name="qkvp", bufs=1))
        rtab_sb = qkvp.tile([128, 2, 2, 256], F32)
        nc.gpsimd.dma_start(rtab_sb[:], rtab.rearrange("s c p t -> p s c t"))
        x_sb = ph1.tile([128, 2, DIM], F32)           # own rows [slotL | slotH]
        for s in range(2):
            eng = nc.sync if s == 0 else nc.scalar
            eng.dma_start(x_sb[:, s, :], x_rows[s])
        # K/V projection weights fully resident so the K AllGather can launch
        # as early as possible (critical path of the whole kernel)
        wkv_ctx = contextlib.ExitStack()
        wkvres = wkv_ctx.enter_context(tc.tile_pool(name="wkvres", bufs=1))
        wkv_sb = wkvres.tile([128, NIC, 1024], BF16)
        _wkv_engs = [nc.gpsimd, nc.scalar, nc.sync, nc.scalar]
        for hf in range(4):
            _wkv_engs[hf].dma_start(
                wkv_sb[:, hf * 4:(hf + 1) * 4, :],
                wqkvT[:, 0:1024].rearrange(
                    "(i p) c -> p i c", p=128)[:, hf * 4:(hf + 1) * 4, :])
        # PE warmup: DMA-independent matmuls raise HAM to K=8/8 while the
        # prologue DMAs/norm run; result sunk to DRAM to stay live.
        warm_sink = dram.tile([128, 1], F32)
        with (
            tc.tile_pool(name="warmp", bufs=1, space="PSUM") as warmp,
            tc.tile_pool(name="warms", bufs=1) as warms,
        ):
            wrm = warms.tile([128, 512], BF16)
            nc.gpsimd.memset(wrm[:], 1.0)
            wps = warmp.tile([128, 512], F32)
            for wi in range(8):
                nc.tensor.matmul(wps[:], ident[:], wrm[:],
                                 start=True, stop=True)
            wsb = warms.tile([128, 1], F32)
            nc.vector.tensor_copy(wsb[:], wps[:, 0:1])
            nc.sync.dma_start(warm_sink[:], wsb[:])
        bias_sb = qkvp.tile([1, 3072], BF16)
        nc.sync.dma_start(bias_sb[:], biases[0:1, :])
        bias_wo = ph1.tile([1, DIM], BF16)
        nc.sync.dma_start(bias_wo[:], biases[1:2, 0:DIM])
        mask_sb = ph1.tile([128, 6, 512], BF16)
        nc.sync.dma_start(mask_sb[:], masks.rearrange("k p q -> p k q"))
        xn = qkvp.tile([128, 2, DIM], BF16)
        for s in range(2):
            _rmsnorm_to(nc, small, xn, x_sb, s, eps_tile, h_sb[:, s, :])
        xnT = qkvp.tile([128, NIC, 256], BF16)

        # ===== phase 2a: K/V projections + rope-k + early AllGathers =====
        # qkv_rows[:, slot, 0:2048]=roped q, [2048:2560]=roped k, [2560:3072]=v
        # wqkvT col order: [k(512) | v(512) | q(2048)]
        # xn transposes are interleaved into the K/V matmul loop (transpose for
        # ic runs ~4 iterations ahead of its consuming matmuls).
        qkv_rows = qkvp.tile([128, 2, 3072], BF16)
        kT_own = qkvp.tile([128, HK, 256], BF16)
        q_roped = ph1.tile([128, HQ, 256], BF16)

        def _xn_transpose(ps1, ic):
            for s in range(2):
                tp = ps1.tile([128, 128], BF16, name="tp_xn", tag="tp_xn")
                nc.tensor.transpose(tp[:], xn[:, s, ic * 128:(ic + 1) * 128],
                                    ident[:])
                nc.vector.tensor_copy(xnT[:, ic, s * 128:(s + 1) * 128], tp[:])

        with (
            tc.tile_pool(name="pkv", bufs=1, space="PSUM") as pkv,
            tc.tile_pool(name="rp", bufs=2) as rp,
        ):
            ps = [pkv.tile([128, 512], F32, name=f"pkv{u}", tag=f"pkv{u}")
                  for u in range(4)]          # (oc, slot): oc0=k, oc1=v
            with tc.tile_pool(name="tp1", bufs=3, space="PSUM") as ps1:
                for ic in range(4):
                    _xn_transpose(ps1, ic)
                for ic in range(NIC):
                    if ic + 4 < NIC:
                        _xn_transpose(ps1, ic + 4)
                    for oi in range(2):
                        for sl in range(2):
                            nc.tensor.matmul(
                                ps[oi * 2 + sl][:],
                                xnT[:, ic, sl * 128:(sl + 1) * 128],
                                wkv_sb[:, ic, oi * 512:(oi + 1) * 512],
                                start=(ic == 0), stop=False)
            for oi in range(2):
                for sl in range(2):
                    nc.tensor.matmul(
                        ps[oi * 2 + sl][:], ones_bf[:, 0:128],
                        bias_sb[:, oi * 512:(oi + 1) * 512],
                        start=False, stop=True)
            # k: rope then transpose + ship + AG (K first: attention scores
            # gate on it); v: plain copy, ship + AG second.
            for sl in range(2):
                _rope_psum(nc, rp, rtab_sb, ps[0 + sl],
                           sl, qkv_rows[:, sl, 2048:2560])
            with tc.tile_pool(name="tpk", bufs=2, space="PSUM") as tpk:
                for sl in range(2):
                    for kh in range(HK):
                        tp = tpk.tile([128, 128], BF16, name="tp_k",
                                      tag="tp_k")
                        nc.tensor.transpose(
                            tp[:],
                            qkv_rows[:, sl,
                                     2048 + kh * 128:2048 + (kh + 1) * 128],
                            ident[:])
                        nc.vector.tensor_copy(
                            kT_own[:, kh, sl * 128:(sl + 1) * 128], tp[:])
            nc.sync.dma_start(
                agk_in.rearrange("(k d n) -> d k n", k=HK, d=128),
                kT_own[:])
            nc.gpsimd.collective_compute(
                "AllGather", ALU.bypass, replica_groups=RG,
                ins=[agk_in.opt()], outs=[agk_out.opt()])
            for sl in range(2):
                nc.vector.tensor_copy(qkv_rows[:, sl, 2560:3072],
                                      ps[2 + sl][:])
            nc.sync.dma_start(
                agv_in.rearrange("(t2 t k) -> t t2 k", t2=2, t=128),
                qkv_rows[:, :, 2560:3072])
            nc.gpsimd.collective_compute(
                "AllGather", ALU.bypass, replica_groups=RG,
                ins=[agv_in.opt()], outs=[agv_out.opt()])

        wkv_ctx.close()
        # ===== phase 2b: Q projections + rope + transposes (overlap AGs) ====
        with (
            tc.tile_pool(name="wq", bufs=2) as wqp,
            tc.tile_pool(name="pq", bufs=1, space="PSUM") as pq,
            tc.tile_pool(name="rp2", bufs=2) as rp2,
        ):
            psq = [pq.tile([128, 512], F32, name=f"pq{u}", tag=f"pq{u}")
                   for u in range(8)]         # (oc, slot)
            for ic in range(NIC):
                w_t = wqp.tile([128, 2048], BF16, name="wq_t", tag="wqt")
                nc.sync.dma_start(
                    w_t[:], wqkvT[ic * 128:(ic + 1) * 128, 1024:3072])
                for oi in range(4):
                    for sl in range(2):
                        nc.tensor.matmul(
                            psq[oi * 2 + sl][:],
                            xnT[:, ic, sl * 128:(sl + 1) * 128],
                            w_t[:, oi * 512:(oi + 1) * 512],
                            start=(ic == 0), stop=False)
            for oi in range(4):
                for sl in range(2):
                    nc.tensor.matmul(
                        psq[oi * 2 + sl][:], ones_bf[:, 0:128],
                        bias_sb[:, 1024 + oi * 512:1024 + (oi + 1) * 512],
                        start=False, stop=True)
            for oi in range(4):
                for sl in range(2):
                    _rope_psum(nc, rp2, rtab_sb, psq[oi * 2 + sl],
                               sl, qkv_rows[:, sl, oi * 512:(oi + 1) * 512])
        # transposes: q -> q_roped [d, h, n] (head-ascending: scores gate on
        # low heads first)
        with tc.tile_pool(name="tpq", bufs=3, space="PSUM") as tpq:
            for h in range(HQ):
                for sl in range(2):
                    tp = tpq.tile([128, 128], BF16, name="tp_q", tag="tp_q")
                    nc.tensor.matmul(tp[:], qkv_rows[:, sl, h * 128:(h + 1) * 128],
                                     ident[:], is_transpose=True)
                    nc.vector.tensor_copy(q_roped[:, h, sl * 128:(sl + 1) * 128], tp[:])
        if dbg:
            nc.sync.dma_start(dbg["dbg_qkv"][:], qkv_rows[:])
            nc.sync.dma_start(dbg["dbg_xn"][:], xn[:])
            nc.sync.dma_start(dbg["dbg_xnT"][:], xnT[:])
        qkv_ctx.close()
        # woT resident for the slot-major wo phase; DMAs run during attention
        wores = att_ctx.enter_context(tc.tile_pool(name="wores", bufs=1))
        woT_sb = wores.tile([128, NIC, DIM], BF16)
        for hf in range(4):
            eng = nc.sync if hf % 2 == 0 else nc.gpsimd
            eng.dma_start(
                woT_sb[:, hf * 4:(hf + 1) * 4, :],
                woT.rearrange("(i p) o -> p i o", p=128)[:, hf * 4:(hf + 1) * 4, :])

        # ============ phase 4: gather K/V into SBUF (rank-major layouts) ============
        # kT_full[:, kh, r, slot*128+t] = rank r's K slot cols; unit code indexes
        # via _owner(j) -> (r, slot).  K gathers first (scores gate on them),
        # spread over 3 queues.
        kT_full = ph1.tile([128, HK, NCORES, 256], BF16)
        v_aug = ph1.tile([128, NCORES, 2, HK, 132], BF16)
        kengs = [nc.sync, nc.gpsimd, nc.scalar]
        for r in range(NCORES):
            kengs[r % 3].dma_start(
                kT_full[:, :, r, :],
                agk_out[r].rearrange("(k d n) -> d k n", k=HK, d=128))
        nc.gpsimd.memset(v_aug[:, :, :, :, 128:129], 1.0)
        for r in range(NCORES):
            vsrc = agv_out[r].rearrange(
                "(t2 t k d) -> t t2 k d", t2=2, t=128, k=HK)
            for sl2 in range(2):
                kengs[r % 3].dma_start(v_aug[:, r, sl2, :, 0:128],
                                       vsrc[:, sl2, :, :])

        # ============ phase 5: attention ============
        # units 0..7 (pairs of s-chunks vs both stripes) packed 2-per-psum-bank;
        # units 8..15 (H-stripe only) packed 4-per-bank. One stt+exp per bank.
        # Score phases are hoisted AV_DEPTH heads ahead of AV phases so the
        # PE keeps scoring while the V AllGather is still in flight, and the
        # vector/scalar mask+exp chain pipelines ahead of the AV matmuls.
        attn = ph1.tile([128, 2, DIM], BF16)     # row-major attn out (normalized)
        attnT = ph1.tile([128, NIC, 256], BF16)
        AV_DEPTH = 3
        with (
            tc.tile_pool(name="ps_sc", bufs=3, space="PSUM") as ps_sc,
            tc.tile_pool(name="ps_av", bufs=2, space="PSUM") as ps_av,
            tc.tile_pool(name="tp_at", bufs=1, space="PSUM") as tp_at,
            tc.tile_pool(name="att_sb", bufs=6 * (AV_DEPTH + 1)) as att_sbp,
            tc.tile_pool(name="att_tmp", bufs=3) as att_tmp,
        ):
            def scores_phase(h):
                kh = h % HK
                att_tiles = []
                for pair in range(4):            # units 2*pair, 2*pair+1
                    sc = ps_sc.tile([128, 512], F32, name="sc", tag="sc")
                    for u in range(2):
                        k = pair * 2 + u
                        rk, sk = _owner(k)
                        nc.tensor.matmul(
                            sc[:, u * 256:(u + 1) * 256],
                            kT_full[:, kh, rk, sk * 128:(sk + 1) * 128],
                            q_roped[:, h, :], start=True, stop=True)
                    tmp = att_tmp.tile([128, 512], BF16, name="mtmp", tag="mtmp")
                    nc.scalar.activation(tmp[:], sc[:], AF.Exp, scale=SCALE)
                    att = att_sbp.tile([128, 512], BF16, name="attP", tag="attP")
                    nc.vector.tensor_mul(att[:], tmp[:], mask_sb[:, pair, :])
                    att_tiles.append(att)
                for quad in range(2):            # units 8..11, 12..15 (H only)
                    sc = ps_sc.tile([128, 512], F32, name="sc", tag="sc")
                    for u in range(4):
                        k = 8 + quad * 4 + u
                        rk, sk = _owner(k)
                        nc.tensor.matmul(
                            sc[:, u * 128:(u + 1) * 128],
                            kT_full[:, kh, rk, sk * 128:(sk + 1) * 128],
                            q_roped[:, h, 128:256], start=True, stop=True)
                    tmp = att_tmp.tile([128, 512], BF16, name="mtmp", tag="mtmp")
                    nc.scalar.activation(tmp[:], sc[:], AF.Exp, scale=SCALE)
                    att = att_sbp.tile([128, 512], BF16, name="attP", tag="attP")
                    nc.vector.tensor_mul(att[:], tmp[:], mask_sb[:, 4 + quad, :])
                    att_tiles.append(att)
                return att_tiles

            def av_phase(h, att_tiles):
                kh = h % HK
                av = [ps_av.tile([128, 132], F32, name=f"av{s}", tag=f"av{s}")
                      for s in range(2)]
                for pair in range(4):
                    att = att_tiles[pair]
                    for u in range(2):
                        k = pair * 2 + u
                        rk, sk = _owner(k)
                        vap = v_aug[:, rk, sk, kh, 0:129]
                        nc.tensor.matmul(
                            av[0][:, 0:129], att[:, u * 256:u * 256 + 128],
                            vap, start=(k == 0), stop=(k == 7))
                        nc.tensor.matmul(
                            av[1][:, 0:129], att[:, u * 256 + 128:u * 256 + 256],
                            vap, start=(k == 0), stop=False)
                for quad in range(2):
                    att = att_tiles[4 + quad]
                    for u in range(4):
                        k = 8 + quad * 4 + u
                        rk, sk = _owner(k)
                        nc.tensor.matmul(
                            av[1][:, 0:129], att[:, u * 128:(u + 1) * 128],
                            v_aug[:, rk, sk, kh, 0:129],
                            start=False, stop=(k == NCH - 1))
                # normalize by denominator (col 128), then transpose this
                # head's column into attnT right away
                for s in range(2):
                    rd = small.tile([128, 1], F32, name="rd", tag="rd")
                    nc.vector.reciprocal(rd[:], av[s][:, 128:129])
                    nc.vector.tensor_scalar_mul(
                        attn[:, s, h * 128:(h + 1) * 128], av[s][:, 0:128], rd[:])
                for s in range(2):
                    tp = tp_at.tile([128, 128], BF16, name="tp_a", tag="tp_a")
                    nc.tensor.transpose(
                        tp[:], attn[:, s, h * 128:(h + 1) * 128], ident[:])
                    nc.vector.tensor_copy(
                        attnT[:, h, s * 128:(s + 1) * 128], tp[:])

            pend = []
            for h in range(HQ):
                pend.append((h, scores_phase(h)))
                if len(pend) > AV_DEPTH:
                    av_phase(*pend.pop(0))
            for item in pend:
                av_phase(*item)

        if dbg:
            nc.sync.dma_start(dbg["dbg_attn"][:], attn[:])

        # ======== phase 6+7: slot-major wo + residual + norm2 + AG ========
        # woT is resident (loaded during attention), so wo runs slot-major:
        # slot L finishes first and its AllGather ships while slot H's wo
        # matmuls are still running — the MLP's L-half gates only on AG-L.
        x2n = ph1.tile([128, 2, DIM], BF16)
        with (
            tc.tile_pool(name="po", bufs=1, space="PSUM") as po,
            tc.tile_pool(name="tp3", bufs=3, space="PSUM") as ps3,
        ):
            for s in range(2):
                pso = [po.tile([128, 512], F32, name=f"pso{i}", tag=f"pso{i}")
                       for i in range(4)]
                for ic in range(NIC):
                    for oc in range(4):
                        nc.tensor.matmul(
                            pso[oc][:],
                            attnT[:, ic, s * 128:(s + 1) * 128],
                            woT_sb[:, ic, oc * 512:(oc + 1) * 512],
                            start=(ic == 0), stop=False)
                for oc in range(4):
                    nc.tensor.matmul(
                        pso[oc][:], ones_bf[:, 0:128],
                        bias_wo[:, oc * 512:(oc + 1) * 512],
                        start=False, stop=True)
                for oc in range(4):
                    nc.vector.tensor_add(
                        h_sb[:, s, oc * 512:(oc + 1) * 512],
                        pso[oc][:], x_sb[:, s, oc * 512:(oc + 1) * 512])
                _rmsnorm_to(nc, small, x2n, h_sb, s, eps_tile, x_sb[:, s, :])
                for ic in range(NIC):
                    tp = ps3.tile([128, 128], BF16, name="tp_x2", tag="tp_x2")
                    nc.tensor.transpose(
                        tp[:], x2n[:, s, ic * 128:(ic + 1) * 128], ident[:])
                    nc.vector.tensor_copy(
                        x2nT[:, ic, s * 128:(s + 1) * 128], tp[:])
                eng = nc.sync if s == 0 else nc.scalar
                eng.dma_start(
                    agx_in[s].rearrange("(i p t) -> p i t", i=NIC, p=128),
                    x2nT[:, :, s * 128:(s + 1) * 128])
                nc.gpsimd.collective_compute(
                    "AllGather", ALU.bypass, replica_groups=RG,
                    ins=[agx_in[s].opt()], outs=[agx_outs[s].opt()])
        if dbg:
            nc.sync.dma_start(dbg["dbg_h"].rearrange("p s d -> p s d"), h_sb[:])

        # ============ phase 8: MLP (TP, FF/8) with chunked RS ============
        # g-chunks 0,1 read only slot-L columns (nbase < 128) so they gate on
        # the slot-L AllGather alone; slot-H arrives while g=0,1 compute.
        att_ctx.close()
        mlpw = ctx.enter_context(tc.tile_pool(name="mlpw", bufs=1, side="right"))
        w2_sb = mlpw.tile([128, FSC, DIM], BF16)
        for hf in range(2):
            eng = nc.gpsimd if hf == 0 else nc.scalar
            eng.dma_start(
                w2_sb[:, hf * 4:(hf + 1) * 4, :],
                w2T.rearrange("(f p) o -> p f o", p=128)[:, hf * 4:(hf + 1) * 4, :])
        mlp = ctx.enter_context(tc.tile_pool(name="mlp", bufs=1, side="right"))
        x2nT_full = mlp.tile([128, NIC, N], BF16)
        for s in range(2):
            for r in range(NCORES):
                kengs[r % 3].dma_start(
                    x2nT_full[:, :, r * 256 + s * 128:r * 256 + (s + 1) * 128],
                    agx_outs[s][r].rearrange("(i p t) -> p i t", i=NIC, p=128))

        w13p = mlpw
        with (
            tc.tile_pool(name="ps_y", bufs=2, space="PSUM") as ps_y,
            tc.tile_pool(name="h2p", bufs=2) as h2p,
            tc.tile_pool(name="ps_w2", bufs=2, space="PSUM") as ps_w2,
            tc.tile_pool(name="rs_sb", bufs=3) as rs_sbp,
        ):
            for g in range(4):             # n-super-chunk = P2 block g
                # rhs columns: P1 cols {r*256 + (g//2)*128 + (g%2)*64 .. +64}
                nbase = (g // 2) * 128 + (g % 2) * 64
                h2T = h2p.tile([128, FSC, 512], BF16, name="h2T", tag="h2T")
                for f in range(FSC):
                    w1_t = w13p.tile([128, NIC, 128], BF16, name="w1_t", tag="w1", bufs=3)
                    nc.sync.dma_start(
                        w1_t.rearrange("p i f -> p (i f)"), w1S[f])
                    w3_t = w13p.tile([128, NIC, 128], BF16, name="w3_t", tag="w3", bufs=3)
                    nc.gpsimd.dma_start(
                        w3_t.rearrange("p i f -> p (i f)"), w3S[f])
                    y1 = ps_y.tile([128, 512], F32, name="y1", tag="y1")
                    y3 = ps_y.tile([128, 512], F32, name="y3", tag="y3")
                    for ic in range(NIC):
                        rhs = x2nT_full[:, ic, :].rearrange(
                            "p (r t) -> p r t", t=256)[:, :, nbase:nbase + 64]
                        nc.tensor.matmul(y1[:], w1_t[:, ic, :], rhs,
                                         start=(ic == 0), stop=(ic == NIC - 1))
                        nc.tensor.matmul(y3[:], w3_t[:, ic, :], rhs,
                                         start=(ic == 0), stop=(ic == NIC - 1))
                    sg = rs_sbp.tile([128, 512], BF16, name="sg", tag="sg")
                    nc.scalar.activation(sg[:], y1[:], AF.Sigmoid)
                    sil = rs_sbp.tile([128, 512], F32, name="sil", tag="sil")
                    nc.vector.scalar_tensor_tensor(
                        sil[:], y1[:], 1.0, sg[:], op0=ALU.mult, op1=ALU.mult)
                    nc.vector.tensor_mul(h2T[:, f, :], sil[:], y3[:])
                # w2: out partial rows for P2 block g
                for q in range(4):         # 128-row slices within block
                    for oc in range(4):
                        pw = ps_w2.tile([128, 512], F32, name="pw", tag="pw")
                        for f in range(FSC):
                            nc.tensor.matmul(
                                pw[:], h2T[:, f, q * 128:(q + 1) * 128],
                                w2_sb[:, f, oc * 512:(oc + 1) * 512],
                                start=(f == 0), stop=(f == FSC - 1))
                        ob = rs_sbp.tile([128, 512], BF16, name="ob", tag="ob")
                        if (q * 4 + oc) % 2 == 0:
                            nc.vector.tensor_copy(ob[:], pw[:])
                        else:
                            nc.scalar.copy(ob[:], pw[:])
                        nc.sync.dma_start(
                            rs_in[g, q * 128:(q + 1) * 128,
                                  oc * 512:(oc + 1) * 512], ob[:])
                nc.gpsimd.collective_compute(
                    "ReduceScatter", ALU.add, replica_groups=RG,
                    ins=[rs_in[g].opt()], outs=[rs_out[g].opt()])

        # ============ phase 9: final residual + output (per RS chunk) ============
        rs_res = mlp.tile([128, 2, DIM], BF16)
        out_sb = mlp.tile([128, 2, DIM], F32)
        for g in range(4):
            s, half = g // 2, g % 2
            pr = slice(half * 64, (half + 1) * 64)
            eng = nc.sync if g % 2 == 0 else nc.gpsimd
            eng.dma_start(rs_res[pr, s, :], rs_out[g])
            nc.vector.tensor_add(out_sb[pr, s, :], rs_res[pr, s, :], h_sb[pr, s, :])
            eng.dma_start(out_ext[s, pr, :], out_sb[pr, s, :])


# ============================ host side ============================

def _perm(nheads):
    p = []
    for h in range(nheads):
        base = h * HD
        p.extend(range(base, base + HD, 2))
        p.extend(range(base + 1, base + HD, 2))
    return np.array(p)


def _rope_tabs(pos):
    inv = 1.0 / (ROPE_BASE ** (np.arange(0, HD, 2, dtype=np.float32) / HD))
    f = np.outer(pos.astype(np.float32), inv)        # [n, 64]
    return np.cos(f).T.astype(np.float32), np.sin(f).T.astype(np.float32)


def _mask_for(stripe, j):
    """multiplicative mask [128 s, 128 q] for s-chunk j vs q-stripe `stripe`"""
    if j < stripe:
        return np.ones((128, 128), np.float32)
    if j > stripe:
        return np.zeros((128, 128), np.float32)
    i = np.arange(128)
    return np.where(i[:, None] <= i[None, :], 1.0, 0.0).astype(np.float32)


def _wimg(wshard):
    """[1024, 2048] w-shard -> SBUF images [8 f-chunks, 128 part(i%128), 16*128]
    img[f][p, ic*128+t] = w.T[ic*128+p, f*128+t]"""
    wT = wshard.T                      # [2048 i, 1024 f]
    img = wT.reshape(NIC, 128, FSC, 128).transpose(2, 1, 0, 3).reshape(FSC, 128, DIM)
    return np.ascontiguousarray(img).astype(BF)


_CACHED_NC = None


def _get_nc():
    global _CACHED_NC
    if _CACHED_NC is None:
        _CACHED_NC = _build_kernel()
    return _CACHED_NC


def _prep_in_maps(inputs):
    f32 = lambda a: np.ascontiguousarray(np.asarray(a), dtype=np.float32)
    x = f32(inputs["x"])[0]                  # [N, DIM]
    g_attn, g_mlp = f32(inputs["g_attn"]), f32(inputs["g_mlp"])
    pq, pk = _perm(HQ), _perm(HK)
    wq = f32(inputs["wq"])[pq] * g_attn[None, :]
    wk = f32(inputs["wk"])[pk] * g_attn[None, :]
    wv = f32(inputs["wv"]) * g_attn[None, :]
    wo = f32(inputs["wo"])
    w1 = f32(inputs["w1"]) * g_mlp[None, :]
    w3 = f32(inputs["w3"]) * g_mlp[None, :]
    w2 = f32(inputs["w2"])
    biases = np.zeros((2, 3072), np.float32)
    biases[0, 0:KV] = f32(inputs["bk"])[pk]
    biases[0, KV:2 * KV] = f32(inputs["bv"])
    biases[0, 2 * KV:] = f32(inputs["bq"])[pq]
    biases[1, 0:DIM] = f32(inputs["bo"])

    wqkv = np.concatenate([wk, wv, wq], 0)         # [3072, 2048] (k|v|q)
    shared = {
        "wqkvT": np.ascontiguousarray(wqkv.T).astype(BF),
        "woT": np.ascontiguousarray(wo.T).astype(BF),
        "biases": biases.astype(BF),
    }
    in_maps = []
    for c in range(NCORES):
        sl, sh = c, NCH - 1 - c
        pos = np.concatenate([np.arange(sl * 128, (sl + 1) * 128),
                              np.arange(sh * 128, (sh + 1) * 128)])
        cos, sin = _rope_tabs(pos)           # [64, 256] feature-major
        # row-major per-slot tables tiled 4x along free: [2 slot, 2 (cos,sin), 128, 256]
        rt = np.zeros((2, 2, 128, 256), np.float32)
        for slot_i in range(2):
            cr = cos[:, slot_i * 128:(slot_i + 1) * 128].T    # [128, 64]
            sr = sin[:, slot_i * 128:(slot_i + 1) * 128].T
            rt[slot_i, 0] = np.tile(cr, (1, 4))
            rt[slot_i, 1] = np.tile(sr, (1, 4))
        # masks [6, 128, 512]: 4 pair-tiles (units 0..7, [L|H0|L|H0]) then
        # 2 quad-tiles (units 8..15, H-stripe only)
        m = np.zeros((6, 128, 512), np.float32)
        for p_ in range(4):
            m[p_, :, 0:128] = _mask_for(sl, 2 * p_)
            m[p_, :, 256:384] = _mask_for(sl, 2 * p_ + 1)
            # H-stripe columns of pair units: chunks 0..7 are always visible
            m[p_, :, 128:256] = 1.0
            m[p_, :, 384:512] = 1.0
        for q_ in range(2):
            for u_ in range(4):
                m[4 + q_, :, u_ * 128:(u_ + 1) * 128] = _mask_for(sh, 8 + q_ * 4 + u_)
        im = {
            "x_rows": np.stack([x[sl * 128:(sl + 1) * 128],
                                x[sh * 128:(sh + 1) * 128]]),
            "rtab": rt,
            "masks": m.astype(BF),
            "biases": shared["biases"],
            "wqkvT": shared["wqkvT"], "woT": shared["woT"],
            "w1S": _wimg(w1[c * FSH:(c + 1) * FSH]),
            "w3S": _wimg(w3[c * FSH:(c + 1) * FSH]),
            "w2T": np.ascontiguousarray(
                w2[:, c * FSH:(c + 1) * FSH].T).astype(BF),
        }
        in_maps.append(im)
    return in_maps


def kernel(**inputs) -> np.ndarray:
    nc = _get_nc()
    in_maps = _prep_in_maps(inputs)
    res = run_bass_kernel_spmd(nc, in_maps, core_ids=list(range(NCORES)))
    out = np.empty((1, N, DIM), np.float32)
    for c in range(NCORES):
        o = res.results[c]["out"]            # [2, 128, DIM]
        out[0, c * 128:(c + 1) * 128] = o[0]
        out[0, (NCH - 1 - c) * 128:(NCH - c) * 128] = o[1]
    return out



# revision 35
# speedup vs baseline: 1.1163x; 1.0276x over previous
"""Fused pre-norm decoder layer (RMSNorm + GQA causal attention w/ RoPE +
RMSNorm + SwiGLU MLP) on 8 Trainium2 NeuronCores.

Sharding: sequence-parallel with folded stripe pairs — core c owns row stripes
{c, 15-c} (128 rows each) so causal attention work is balanced; the MLP is
tensor-parallel (w1/w3 column-split, w2 row-split). Cross-core comms:
AllGather of roped K^T + V (bf16), AllGather of the transposed normed hidden
states (bf16), and a 4-way chunked ReduceScatter of the MLP partial outputs
(bf16) overlapped with the w2 matmuls. Matmuls in bf16, residuals in fp32.

Self-contained: hardcodes the reference shapes
(B=1, N=2048, DIM=2048, HQ=16, HK=4, HD=128, F=8192).
"""
import numpy as np
import ml_dtypes

import concourse.bass as bass
import concourse.mybir as mybir
import concourse.tile as tile
from concourse import bacc
from concourse.bass_utils import run_bass_kernel_spmd
from concourse.masks import make_identity

F32 = mybir.dt.float32
BF16 = mybir.dt.bfloat16
AF = mybir.ActivationFunctionType
ALU = mybir.AluOpType
BF = ml_dtypes.bfloat16

DIM = 2048
HQ = 16            # query heads
HK = 4             # kv heads
HD = 128           # head dim
KV = HD * HK       # 512
N = 2048           # sequence length
FF = 4 * DIM       # 8192 mlp hidden
EPS = 1e-6
ROPE_BASE = 10000.0
SCALE = HD ** -0.5

NCORES = 8
RG = [list(range(NCORES))]
NCH = N // 128       # 16 sequence chunks
NIC = DIM // 128     # 16 feature chunks
FSH = FF // NCORES   # 1024 mlp hidden per core
FSC = FSH // 128     # 8 f-chunks per core
NEG = -1e30
DEBUG = False

# core c owns stripes (c, 15-c); local rows = [stripe_c | stripe_{15-c}]
# global s-chunk j lives on core own(j), slot slot(j):
def _owner(j):
    return (j, 0) if j < NCH // 2 else (NCH - 1 - j, 1)


def _build_kernel():
    nc = bacc.Bacc(None, target_bir_lowering=False)

    x_rows = nc.dram_tensor("x_rows", [2, 128, DIM], F32, kind="ExternalInput")
    rtab = nc.dram_tensor("rtab", [2, 2, 128, 256], F32, kind="ExternalInput")
    masks = nc.dram_tensor("masks", [6, 128, 512], BF16, kind="ExternalInput")
    biases = nc.dram_tensor("biases", [2, 3072], BF16, kind="ExternalInput")
    wqkvT = nc.dram_tensor("wqkvT", [DIM, 3072], BF16, kind="ExternalInput")
    woT = nc.dram_tensor("woT", [DIM, DIM], BF16, kind="ExternalInput")
    w1S = nc.dram_tensor("w1S", [FSC, 128, DIM], BF16, kind="ExternalInput")
    w3S = nc.dram_tensor("w3S", [FSC, 128, DIM], BF16, kind="ExternalInput")
    w2T = nc.dram_tensor("w2T", [FSH, DIM], BF16, kind="ExternalInput")
    out_ext = nc.dram_tensor("out", [2, 128, DIM], F32, kind="ExternalOutput")
    dbg = {}
    if DEBUG:
        for nm, shp, dt in [("dbg_qkv", [128, 2, 3072], BF16),
                            ("dbg_attn", [128, 2, DIM], BF16),
                            ("dbg_h", [128, 2, DIM], F32),
                            ("dbg_xn", [128, 2, DIM], BF16),
                            ("dbg_xnT", [128, NIC, 256], BF16)]:
            dbg[nm] = nc.dram_tensor(nm, shp, dt, kind="ExternalOutput")

    with tile.TileContext(nc) as tc:
        _body(nc, tc, x_rows, rtab, masks, biases,
              wqkvT, woT, w1S, w3S, w2T, out_ext, dbg)
    nc.compile()
    return nc


def _rope_psum(nc, rp, rtab_sb, pcur, sl, dst):
    """rope a [128, 512] psum tile (4 head-blocks) into dst [128, 512] bf16."""
    AFv = mybir.ActivationFunctionType
    pv = pcur.rearrange("p (h t) -> p h t", t=128)
    cosT = rtab_sb[:, sl, 0, :].rearrange("p (h t) -> p h t", t=64)
    sinT = rtab_sb[:, sl, 1, :].rearrange("p (h t) -> p h t", t=64)
    t1 = rp.tile([128, 4, 64], F32, name="t1", tag="t1")
    t2 = rp.tile([128, 4, 64], F32, name="t2", tag="t2")
    t3 = rp.tile([128, 4, 64], F32, name="t3", tag="t3")
    t4 = rp.tile([128, 4, 64], F32, name="t4", tag="t4")
    nc.vector.tensor_mul(t1[:], pv[:, :, 0:64], cosT)
    nc.vector.tensor_mul(t2[:], pv[:, :, 64:128], sinT)
    nc.vector.tensor_mul(t3[:], pv[:, :, 0:64], sinT)
    nc.vector.tensor_mul(t4[:], pv[:, :, 64:128], cosT)
    dstv = dst.rearrange("p (h t) -> p h t", t=128)
    nc.vector.tensor_sub(dstv[:, :, 0:64], t1[:], t2[:])
    nc.vector.tensor_add(dstv[:, :, 64:128], t3[:], t4[:])


def _rmsnorm_to(nc, pool, out_bf, x_sb, slot, eps_tile, scratch):
    """out_bf[:, slot, :] = rmsnorm(x_sb[:, slot, :]) cast bf16.
    scratch: any writable [128, DIM] f32 AP whose contents may be clobbered."""
    ssq = pool.tile([128, 1], F32, name="ssq", tag="ssq")
    nc.scalar.activation(scratch, x_sb[:, slot, :], AF.Square, accum_out=ssq[:])
    rms = pool.tile([128, 1], F32, name="rms", tag="rms")
    nc.scalar.activation(rms[:], ssq[:], AF.Sqrt, bias=eps_tile[:], scale=1.0 / DIM)
    rinv = pool.tile([128, 1], F32, name="rinv", tag="rinv")
    nc.vector.reciprocal(rinv[:], rms[:])
    nc.vector.tensor_scalar_mul(out_bf[:, slot, :], x_sb[:, slot, :], rinv[:])


def _transpose_2x16(nc, sb, ps, dst, src, ident, tag):
    """src [128, 2, 2048] bf16 row-major -> dst [128, 16, 256] bf16 (feature-major).
    dst[:, ic, s*128:(s+1)*128] = src[:, s, ic*128:(ic+1)*128].T
    """
    for s in range(2):
        for ic in range(NIC):
            tp = ps.tile([128, 128], BF16, name=f"tp_{tag}", tag=f"tp_{tag}")
            nc.tensor.transpose(tp[:], src[:, s, ic * 128:(ic + 1) * 128], ident[:])
            nc.vector.tensor_copy(dst[:, ic, s * 128:(s + 1) * 128], tp[:])


def _body(nc, tc, x_rows, rtab, masks, biases,
          wqkvT, woT, w1S, w3S, w2T, out_ext, dbg={}):
    import contextlib
    ctx = contextlib.ExitStack()
    with ctx:
        const = ctx.enter_context(tc.tile_pool(name="const", bufs=1))
        persist = ctx.enter_context(tc.tile_pool(name="persist", bufs=1))
        dram = ctx.enter_context(tc.tile_pool(name="dram", bufs=1, space="DRAM"))
        small = ctx.enter_context(tc.tile_pool(name="small", bufs=4))

        ident = const.tile([128, 128], BF16)
        make_identity(nc, ident)
        eps_tile = const.tile([128, 1], F32)
        nc.gpsimd.memset(eps_tile[:], EPS)
        ones_bf = const.tile([1, 512], BF16)
        nc.gpsimd.memset(ones_bf[:], 1.0)
        bias_sb = None  # allocated in attention pool below

        # DRAM comm buffers
        agk_in = dram.tile([HK * 128 * 256], BF16)
        agk_out = dram.tile([NCORES, HK * 128 * 256], BF16,
                            addr_space="Shared")
        agv_in = dram.tile([2 * 128 * KV], BF16)
        agv_out = dram.tile([NCORES, 2 * 128 * KV], BF16,
                            addr_space="Shared")
        agx_in = dram.tile([2, NIC * 128 * 128], BF16)
        agx_outs = [dram.tile([NCORES, NIC * 128 * 128], BF16,
                              addr_space="Shared", name=f"agxo{s_}")
                    for s_ in range(2)]
        rs_in = dram.tile([4, 512, DIM], BF16)
        rs_out = dram.tile([4, 64, DIM], BF16)

        # persistent SBUF
        h_sb = persist.tile([128, 2, DIM], F32)       # post-attention residual
        x2nT = persist.tile([128, NIC, 256], BF16)

        # attention-phase pool: closed before the MLP pool allocates
        att_ctx = contextlib.ExitStack()
        ph1 = att_ctx.enter_context(tc.tile_pool(name="ph1", bufs=1))
        qkv_ctx = contextlib.ExitStack()
        qkvp = qkv_ctx.enter_context(tc.tile_pool(name="qkvp", bufs=1))
        rtab_sb = qkvp.tile([128, 2, 2, 256], F32)
        nc.gpsimd.dma_start(rtab_sb[:], rtab.rearrange("s c p t -> p s c t"))
        x_sb = ph1.tile([128, 2, DIM], F32)           # own rows [slotL | slotH]
        for s in range(2):
            eng = nc.sync if s == 0 else nc.scalar
            eng.dma_start(x_sb[:, s, :], x_rows[s])
        # K/V projection weights fully resident so the K AllGather can launch
        # as early as possible (critical path of the whole kernel)
        wkv_ctx = contextlib.ExitStack()
        wkvres = wkv_ctx.enter_context(tc.tile_pool(name="wkvres", bufs=1))
        wkv_sb = wkvres.tile([128, NIC, 1024], BF16)
        _wkv_engs = [nc.gpsimd, nc.scalar, nc.sync, nc.scalar]
        for hf in range(4):
            _wkv_engs[hf].dma_start(
                wkv_sb[:, hf * 4:(hf + 1) * 4, :],
                wqkvT[:, 0:1024].rearrange(
                    "(i p) c -> p i c", p=128)[:, hf * 4:(hf + 1) * 4, :])
        # PE warmup: DMA-independent matmuls raise HAM to K=8/8 while the
        # prologue DMAs/norm run; result sunk to DRAM to stay live.
        warm_sink = dram.tile([128, 1], F32)
        with (
            tc.tile_pool(name="warmp", bufs=1, space="PSUM") as warmp,
            tc.tile_pool(name="warms", bufs=1) as warms,
        ):
            wrm = warms.tile([128, 512], BF16)
            nc.gpsimd.memset(wrm[:], 1.0)
            wps = warmp.tile([128, 512], F32)
            for wi in range(8):
                nc.tensor.matmul(wps[:], ident[:], wrm[:],
                                 start=True, stop=True)
            wsb = warms.tile([128, 1], F32)
            nc.vector.tensor_copy(wsb[:], wps[:, 0:1])
            nc.sync.dma_start(warm_sink[:], wsb[:])
        bias_sb = qkvp.tile([1, 3072], BF16)
        nc.sync.dma_start(bias_sb[:], biases[0:1, :])
        bias_wo = ph1.tile([1, DIM], BF16)
        nc.sync.dma_start(bias_wo[:], biases[1:2, 0:DIM])
        mask_sb = ph1.tile([128, 6, 512], BF16)
        nc.sync.dma_start(mask_sb[:], masks.rearrange("k p q -> p k q"))
        xn = qkvp.tile([128, 2, DIM], BF16)
        for s in range(2):
            _rmsnorm_to(nc, small, xn, x_sb, s, eps_tile, h_sb[:, s, :])
        xnT = qkvp.tile([128, NIC, 256], BF16)

        # ===== phase 2a: K/V projections + rope-k + early AllGathers =====
        # qkv_rows[:, slot, 0:2048]=roped q, [2048:2560]=roped k, [2560:3072]=v
        # wqkvT col order: [k(512) | v(512) | q(2048)]
        # xn transposes are interleaved into the K/V matmul loop (transpose for
        # ic runs ~4 iterations ahead of its consuming matmuls).
        qkv_rows = qkvp.tile([128, 2, 3072], BF16)
        kT_own = qkvp.tile([128, HK, 256], BF16)
        q_roped = ph1.tile([128, HQ, 256], BF16)

        def _xn_transpose(ps1, ic):
            for s in range(2):
                tp = ps1.tile([128, 128], BF16, name="tp_xn", tag="tp_xn")
                nc.tensor.transpose(tp[:], xn[:, s, ic * 128:(ic + 1) * 128],
                                    ident[:])
                nc.vector.tensor_copy(xnT[:, ic, s * 128:(s + 1) * 128], tp[:])

        with (
            tc.tile_pool(name="pkv", bufs=1, space="PSUM") as pkv,
            tc.tile_pool(name="rp", bufs=2) as rp,
        ):
            ps = [pkv.tile([128, 512], F32, name=f"pkv{u}", tag=f"pkv{u}")
                  for u in range(4)]          # (oc, slot): oc0=k, oc1=v
            with tc.tile_pool(name="tp1", bufs=3, space="PSUM") as ps1:
                for ic in range(4):
                    _xn_transpose(ps1, ic)
                for ic in range(NIC):
                    if ic + 4 < NIC:
                        _xn_transpose(ps1, ic + 4)
                    for oi in range(2):
                        for sl in range(2):
                            nc.tensor.matmul(
                                ps[oi * 2 + sl][:],
                                xnT[:, ic, sl * 128:(sl + 1) * 128],
                                wkv_sb[:, ic, oi * 512:(oi + 1) * 512],
                                start=(ic == 0), stop=False)
            for oi in range(2):
                for sl in range(2):
                    nc.tensor.matmul(
                        ps[oi * 2 + sl][:], ones_bf[:, 0:128],
                        bias_sb[:, oi * 512:(oi + 1) * 512],
                        start=False, stop=True)
            # k: rope then transpose + ship + AG (K first: attention scores
            # gate on it); v: plain copy, ship + AG second.
            for sl in range(2):
                _rope_psum(nc, rp, rtab_sb, ps[0 + sl],
                           sl, qkv_rows[:, sl, 2048:2560])
            with tc.tile_pool(name="tpk", bufs=2, space="PSUM") as tpk:
                for sl in range(2):
                    for kh in range(HK):
                        tp = tpk.tile([128, 128], BF16, name="tp_k",
                                      tag="tp_k")
                        nc.tensor.transpose(
                            tp[:],
                            qkv_rows[:, sl,
                                     2048 + kh * 128:2048 + (kh + 1) * 128],
                            ident[:])
                        nc.vector.tensor_copy(
                            kT_own[:, kh, sl * 128:(sl + 1) * 128], tp[:])
            nc.sync.dma_start(
                agk_in.rearrange("(k d n) -> d k n", k=HK, d=128),
                kT_own[:])
            nc.gpsimd.collective_compute(
                "AllGather", ALU.bypass, replica_groups=RG,
                ins=[agk_in.opt()], outs=[agk_out.opt()])
            for sl in range(2):
                nc.vector.tensor_copy(qkv_rows[:, sl, 2560:3072],
                                      ps[2 + sl][:])
            nc.sync.dma_start(
                agv_in.rearrange("(t2 t k) -> t t2 k", t2=2, t=128),
                qkv_rows[:, :, 2560:3072])
            nc.gpsimd.collective_compute(
                "AllGather", ALU.bypass, replica_groups=RG,
                ins=[agv_in.opt()], outs=[agv_out.opt()])

        wkv_ctx.close()
        # ===== phase 2b: Q projections + rope + transposes (overlap AGs) ====
        with (
            tc.tile_pool(name="wq", bufs=3) as wqp,
            tc.tile_pool(name="pq", bufs=1, space="PSUM") as pq,
            tc.tile_pool(name="rp2", bufs=2) as rp2,
        ):
            psq = [pq.tile([128, 512], F32, name=f"pq{u}", tag=f"pq{u}")
                   for u in range(8)]         # (oc, slot)
            for ic in range(NIC):
                w_t = wqp.tile([128, 2048], BF16, name="wq_t", tag="wqt")
                eng = nc.sync if ic % 2 == 0 else nc.scalar
                eng.dma_start(
                    w_t[:], wqkvT[ic * 128:(ic + 1) * 128, 1024:3072])
                for oi in range(4):
                    for sl in range(2):
                        nc.tensor.matmul(
                            psq[oi * 2 + sl][:],
                            xnT[:, ic, sl * 128:(sl + 1) * 128],
                            w_t[:, oi * 512:(oi + 1) * 512],
                            start=(ic == 0), stop=False)
            for oi in range(4):
                for sl in range(2):
                    nc.tensor.matmul(
                        psq[oi * 2 + sl][:], ones_bf[:, 0:128],
                        bias_sb[:, 1024 + oi * 512:1024 + (oi + 1) * 512],
                        start=False, stop=True)
            for oi in range(4):
                for sl in range(2):
                    _rope_psum(nc, rp2, rtab_sb, psq[oi * 2 + sl],
                               sl, qkv_rows[:, sl, oi * 512:(oi + 1) * 512])
        # transposes: q -> q_roped [d, h, n] (head-ascending: scores gate on
        # low heads first)
        with tc.tile_pool(name="tpq", bufs=3, space="PSUM") as tpq:
            for h in range(HQ):
                for sl in range(2):
                    tp = tpq.tile([128, 128], BF16, name="tp_q", tag="tp_q")
                    nc.tensor.matmul(tp[:], qkv_rows[:, sl, h * 128:(h + 1) * 128],
                                     ident[:], is_transpose=True)
                    nc.vector.tensor_copy(q_roped[:, h, sl * 128:(sl + 1) * 128], tp[:])
        if dbg:
            nc.sync.dma_start(dbg["dbg_qkv"][:], qkv_rows[:])
            nc.sync.dma_start(dbg["dbg_xn"][:], xn[:])
            nc.sync.dma_start(dbg["dbg_xnT"][:], xnT[:])
        qkv_ctx.close()
        # woT resident for the slot-major wo phase; DMAs run during attention
        wores = att_ctx.enter_context(tc.tile_pool(name="wores", bufs=1))
        woT_sb = wores.tile([128, NIC, DIM], BF16)
        for hf in range(4):
            eng = nc.sync if hf % 2 == 0 else nc.gpsimd
            eng.dma_start(
                woT_sb[:, hf * 4:(hf + 1) * 4, :],
                woT.rearrange("(i p) o -> p i o", p=128)[:, hf * 4:(hf + 1) * 4, :])

        # ============ phase 4: gather K/V into SBUF (rank-major layouts) ============
        # kT_full[:, kh, r, slot*128+t] = rank r's K slot cols; unit code indexes
        # via _owner(j) -> (r, slot).  K gathers first (scores gate on them),
        # spread over 3 queues.
        kT_full = ph1.tile([128, HK, NCORES, 256], BF16)
        v_aug = ph1.tile([128, NCORES, 2, HK, 132], BF16)
        kengs = [nc.sync, nc.gpsimd, nc.scalar]
        for r in range(NCORES):
            kengs[r % 3].dma_start(
                kT_full[:, :, r, :],
                agk_out[r].rearrange("(k d n) -> d k n", k=HK, d=128))
        nc.gpsimd.memset(v_aug[:, :, :, :, 128:129], 1.0)
        for r in range(NCORES):
            vsrc = agv_out[r].rearrange(
                "(t2 t k d) -> t t2 k d", t2=2, t=128, k=HK)
            for sl2 in range(2):
                kengs[r % 3].dma_start(v_aug[:, r, sl2, :, 0:128],
                                       vsrc[:, sl2, :, :])

        # ============ phase 5: attention ============
        # units 0..7 (pairs of s-chunks vs both stripes) packed 2-per-psum-bank;
        # units 8..15 (H-stripe only) packed 4-per-bank. One stt+exp per bank.
        # Score phases are hoisted AV_DEPTH heads ahead of AV phases so the
        # PE keeps scoring while the V AllGather is still in flight, and the
        # vector/scalar mask+exp chain pipelines ahead of the AV matmuls.
        attn = ph1.tile([128, 2, DIM], BF16)     # row-major attn out (normalized)
        attnT = ph1.tile([128, NIC, 256], BF16)
        AV_DEPTH = 3
        with (
            tc.tile_pool(name="ps_sc", bufs=3, space="PSUM") as ps_sc,
            tc.tile_pool(name="ps_av", bufs=2, space="PSUM") as ps_av,
            tc.tile_pool(name="tp_at", bufs=1, space="PSUM") as tp_at,
            tc.tile_pool(name="att_sb", bufs=6 * (AV_DEPTH + 1)) as att_sbp,
            tc.tile_pool(name="att_tmp", bufs=3) as att_tmp,
        ):
            def scores_phase(h):
                kh = h % HK
                att_tiles = []
                for pair in range(4):            # units 2*pair, 2*pair+1
                    sc = ps_sc.tile([128, 512], F32, name="sc", tag="sc")
                    for u in range(2):
                        k = pair * 2 + u
                        rk, sk = _owner(k)
                        nc.tensor.matmul(
                            sc[:, u * 256:(u + 1) * 256],
                            kT_full[:, kh, rk, sk * 128:(sk + 1) * 128],
                            q_roped[:, h, :], start=True, stop=True)
                    tmp = att_tmp.tile([128, 512], BF16, name="mtmp", tag="mtmp")
                    nc.scalar.activation(tmp[:], sc[:], AF.Exp, scale=SCALE)
                    att = att_sbp.tile([128, 512], BF16, name="attP", tag="attP")
                    nc.vector.tensor_mul(att[:], tmp[:], mask_sb[:, pair, :])
                    att_tiles.append(att)
                for quad in range(2):            # units 8..11, 12..15 (H only)
                    sc = ps_sc.tile([128, 512], F32, name="sc", tag="sc")
                    for u in range(4):
                        k = 8 + quad * 4 + u
                        rk, sk = _owner(k)
                        nc.tensor.matmul(
                            sc[:, u * 128:(u + 1) * 128],
                            kT_full[:, kh, rk, sk * 128:(sk + 1) * 128],
                            q_roped[:, h, 128:256], start=True, stop=True)
                    tmp = att_tmp.tile([128, 512], BF16, name="mtmp", tag="mtmp")
                    nc.scalar.activation(tmp[:], sc[:], AF.Exp, scale=SCALE)
                    att = att_sbp.tile([128, 512], BF16, name="attP", tag="attP")
                    nc.vector.tensor_mul(att[:], tmp[:], mask_sb[:, 4 + quad, :])
                    att_tiles.append(att)
                return att_tiles

            def av_phase(h, att_tiles):
                kh = h % HK
                av = [ps_av.tile([128, 132], F32, name=f"av{s}", tag=f"av{s}")
                      for s in range(2)]
                for pair in range(4):
                    att = att_tiles[pair]
                    for u in range(2):
                        k = pair * 2 + u
                        rk, sk = _owner(k)
                        vap = v_aug[:, rk, sk, kh, 0:129]
                        nc.tensor.matmul(
                            av[0][:, 0:129], att[:, u * 256:u * 256 + 128],
                            vap, start=(k == 0), stop=(k == 7))
                        nc.tensor.matmul(
                            av[1][:, 0:129], att[:, u * 256 + 128:u * 256 + 256],
                            vap, start=(k == 0), stop=False)
                for quad in range(2):
                    att = att_tiles[4 + quad]
                    for u in range(4):
                        k = 8 + quad * 4 + u
                        rk, sk = _owner(k)
                        nc.tensor.matmul(
                            av[1][:, 0:129], att[:, u * 128:(u + 1) * 128],
                            v_aug[:, rk, sk, kh, 0:129],
                            start=False, stop=(k == NCH - 1))
                # normalize by denominator (col 128), then transpose this
                # head's column into attnT right away
                for s in range(2):
                    rd = small.tile([128, 1], F32, name="rd", tag="rd")
                    nc.vector.reciprocal(rd[:], av[s][:, 128:129])
                    nc.vector.tensor_scalar_mul(
                        attn[:, s, h * 128:(h + 1) * 128], av[s][:, 0:128], rd[:])
                for s in range(2):
                    tp = tp_at.tile([128, 128], BF16, name="tp_a", tag="tp_a")
                    nc.tensor.transpose(
                        tp[:], attn[:, s, h * 128:(h + 1) * 128], ident[:])
                    nc.vector.tensor_copy(
                        attnT[:, h, s * 128:(s + 1) * 128], tp[:])

            pend = []
            for h in range(HQ):
                pend.append((h, scores_phase(h)))
                if len(pend) > AV_DEPTH:
                    av_phase(*pend.pop(0))
            for item in pend:
                av_phase(*item)

        if dbg:
            nc.sync.dma_start(dbg["dbg_attn"][:], attn[:])

        # ======== phase 6+7: slot-major wo + residual + norm2 + AG ========
        # woT is resident (loaded during attention), so wo runs slot-major:
        # slot L finishes first and its AllGather ships while slot H's wo
        # matmuls are still running — the MLP's L-half gates only on AG-L.
        x2n = ph1.tile([128, 2, DIM], BF16)
        with (
            tc.tile_pool(name="po", bufs=1, space="PSUM") as po,
            tc.tile_pool(name="tp3", bufs=3, space="PSUM") as ps3,
        ):
            for s in range(2):
                pso = [po.tile([128, 512], F32, name=f"pso{i}", tag=f"pso{i}")
                       for i in range(4)]
                for ic in range(NIC):
                    for oc in range(4):
                        nc.tensor.matmul(
                            pso[oc][:],
                            attnT[:, ic, s * 128:(s + 1) * 128],
                            woT_sb[:, ic, oc * 512:(oc + 1) * 512],
                            start=(ic == 0), stop=False)
                for oc in range(4):
                    nc.tensor.matmul(
                        pso[oc][:], ones_bf[:, 0:128],
                        bias_wo[:, oc * 512:(oc + 1) * 512],
                        start=False, stop=True)
                for oc in range(4):
                    nc.vector.tensor_add(
                        h_sb[:, s, oc * 512:(oc + 1) * 512],
                        pso[oc][:], x_sb[:, s, oc * 512:(oc + 1) * 512])
                _rmsnorm_to(nc, small, x2n, h_sb, s, eps_tile, x_sb[:, s, :])
                for ic in range(NIC):
                    tp = ps3.tile([128, 128], BF16, name="tp_x2", tag="tp_x2")
                    nc.tensor.transpose(
                        tp[:], x2n[:, s, ic * 128:(ic + 1) * 128], ident[:])
                    nc.vector.tensor_copy(
                        x2nT[:, ic, s * 128:(s + 1) * 128], tp[:])
                eng = nc.sync if s == 0 else nc.scalar
                eng.dma_start(
                    agx_in[s].rearrange("(i p t) -> p i t", i=NIC, p=128),
                    x2nT[:, :, s * 128:(s + 1) * 128])
                nc.gpsimd.collective_compute(
                    "AllGather", ALU.bypass, replica_groups=RG,
                    ins=[agx_in[s].opt()], outs=[agx_outs[s].opt()])
        if dbg:
            nc.sync.dma_start(dbg["dbg_h"].rearrange("p s d -> p s d"), h_sb[:])

        # ============ phase 8: MLP (TP, FF/8) with chunked RS ============
        # g-chunks 0,1 read only slot-L columns (nbase < 128) so they gate on
        # the slot-L AllGather alone; slot-H arrives while g=0,1 compute.
        att_ctx.close()
        mlpw = ctx.enter_context(tc.tile_pool(name="mlpw", bufs=1, side="right"))
        w2_sb = mlpw.tile([128, FSC, DIM], BF16)
        for hf in range(2):
            eng = nc.gpsimd if hf == 0 else nc.scalar
            eng.dma_start(
                w2_sb[:, hf * 4:(hf + 1) * 4, :],
                w2T.rearrange("(f p) o -> p f o", p=128)[:, hf * 4:(hf + 1) * 4, :])
        mlp = ctx.enter_context(tc.tile_pool(name="mlp", bufs=1, side="right"))
        x2nT_full = mlp.tile([128, NIC, N], BF16)
        for s in range(2):
            for r in range(NCORES):
                kengs[r % 3].dma_start(
                    x2nT_full[:, :, r * 256 + s * 128:r * 256 + (s + 1) * 128],
                    agx_outs[s][r].rearrange("(i p t) -> p i t", i=NIC, p=128))

        w13p = mlpw
        with (
            tc.tile_pool(name="ps_y", bufs=2, space="PSUM") as ps_y,
            tc.tile_pool(name="h2p", bufs=2) as h2p,
            tc.tile_pool(name="ps_w2", bufs=2, space="PSUM") as ps_w2,
            tc.tile_pool(name="rs_sb", bufs=3) as rs_sbp,
        ):
            for g in range(4):             # n-super-chunk = P2 block g
                # rhs columns: P1 cols {r*256 + (g//2)*128 + (g%2)*64 .. +64}
                nbase = (g // 2) * 128 + (g % 2) * 64
                h2T = h2p.tile([128, FSC, 512], BF16, name="h2T", tag="h2T")
                for f in range(FSC):
                    w1_t = w13p.tile([128, NIC, 128], BF16, name="w1_t", tag="w1", bufs=4)
                    nc.sync.dma_start(
                        w1_t.rearrange("p i f -> p (i f)"), w1S[f])
                    w3_t = w13p.tile([128, NIC, 128], BF16, name="w3_t", tag="w3", bufs=4)
                    nc.gpsimd.dma_start(
                        w3_t.rearrange("p i f -> p (i f)"), w3S[f])
                    y1 = ps_y.tile([128, 512], F32, name="y1", tag="y1")
                    y3 = ps_y.tile([128, 512], F32, name="y3", tag="y3")
                    for ic in range(NIC):
                        rhs = x2nT_full[:, ic, :].rearrange(
                            "p (r t) -> p r t", t=256)[:, :, nbase:nbase + 64]
                        nc.tensor.matmul(y1[:], w1_t[:, ic, :], rhs,
                                         start=(ic == 0), stop=(ic == NIC - 1))
                        nc.tensor.matmul(y3[:], w3_t[:, ic, :], rhs,
                                         start=(ic == 0), stop=(ic == NIC - 1))
                    sg = rs_sbp.tile([128, 512], BF16, name="sg", tag="sg")
                    nc.scalar.activation(sg[:], y1[:], AF.Sigmoid)
                    sil = rs_sbp.tile([128, 512], F32, name="sil", tag="sil")
                    nc.vector.scalar_tensor_tensor(
                        sil[:], y1[:], 1.0, sg[:], op0=ALU.mult, op1=ALU.mult)
                    nc.vector.tensor_mul(h2T[:, f, :], sil[:], y3[:])
                # w2: out partial rows for P2 block g
                for q in range(4):         # 128-row slices within block
                    for oc in range(4):
                        pw = ps_w2.tile([128, 512], F32, name="pw", tag="pw")
                        for f in range(FSC):
                            nc.tensor.matmul(
                                pw[:], h2T[:, f, q * 128:(q + 1) * 128],
                                w2_sb[:, f, oc * 512:(oc + 1) * 512],
                                start=(f == 0), stop=(f == FSC - 1))
                        ob = rs_sbp.tile([128, 512], BF16, name="ob", tag="ob")
                        if (q * 4 + oc) % 2 == 0:
                            nc.vector.tensor_copy(ob[:], pw[:])
                        else:
                            nc.scalar.copy(ob[:], pw[:])
                        nc.sync.dma_start(
                            rs_in[g, q * 128:(q + 1) * 128,
                                  oc * 512:(oc + 1) * 512], ob[:])
                nc.gpsimd.collective_compute(
                    "ReduceScatter", ALU.add, replica_groups=RG,
                    ins=[rs_in[g].opt()], outs=[rs_out[g].opt()])

        # ============ phase 9: final residual + output (per RS chunk) ============
        rs_res = mlp.tile([128, 2, DIM], BF16)
        out_sb = mlp.tile([128, 2, DIM], F32)
        for g in range(4):
            s, half = g // 2, g % 2
            pr = slice(half * 64, (half + 1) * 64)
            eng = nc.sync if g % 2 == 0 else nc.gpsimd
            eng.dma_start(rs_res[pr, s, :], rs_out[g])
            nc.vector.tensor_add(out_sb[pr, s, :], rs_res[pr, s, :], h_sb[pr, s, :])
            eng.dma_start(out_ext[s, pr, :], out_sb[pr, s, :])


# ============================ host side ============================

def _perm(nheads):
    p = []
    for h in range(nheads):
        base = h * HD
        p.extend(range(base, base + HD, 2))
        p.extend(range(base + 1, base + HD, 2))
    return np.array(p)


def _rope_tabs(pos):
    inv = 1.0 / (ROPE_BASE ** (np.arange(0, HD, 2, dtype=np.float32) / HD))
    f = np.outer(pos.astype(np.float32), inv)        # [n, 64]
    return np.cos(f).T.astype(np.float32), np.sin(f).T.astype(np.float32)


def _mask_for(stripe, j):
    """multiplicative mask [128 s, 128 q] for s-chunk j vs q-stripe `stripe`"""
    if j < stripe:
        return np.ones((128, 128), np.float32)
    if j > stripe:
        return np.zeros((128, 128), np.float32)
    i = np.arange(128)
    return np.where(i[:, None] <= i[None, :], 1.0, 0.0).astype(np.float32)


def _wimg(wshard):
    """[1024, 2048] w-shard -> SBUF images [8 f-chunks, 128 part(i%128), 16*128]
    img[f][p, ic*128+t] = w.T[ic*128+p, f*128+t]"""
    wT = wshard.T                      # [2048 i, 1024 f]
    img = wT.reshape(NIC, 128, FSC, 128).transpose(2, 1, 0, 3).reshape(FSC, 128, DIM)
    return np.ascontiguousarray(img).astype(BF)


_CACHED_NC = None


def _get_nc():
    global _CACHED_NC
    if _CACHED_NC is None:
        _CACHED_NC = _build_kernel()
    return _CACHED_NC


def _prep_in_maps(inputs):
    f32 = lambda a: np.ascontiguousarray(np.asarray(a), dtype=np.float32)
    x = f32(inputs["x"])[0]                  # [N, DIM]
    g_attn, g_mlp = f32(inputs["g_attn"]), f32(inputs["g_mlp"])
    pq, pk = _perm(HQ), _perm(HK)
    wq = f32(inputs["wq"])[pq] * g_attn[None, :]
    wk = f32(inputs["wk"])[pk] * g_attn[None, :]
    wv = f32(inputs["wv"]) * g_attn[None, :]
    wo = f32(inputs["wo"])
    w1 = f32(inputs["w1"]) * g_mlp[None, :]
    w3 = f32(inputs["w3"]) * g_mlp[None, :]
    w2 = f32(inputs["w2"])
    biases = np.zeros((2, 3072), np.float32)
    biases[0, 0:KV] = f32(inputs["bk"])[pk]
    biases[0, KV:2 * KV] = f32(inputs["bv"])
    biases[0, 2 * KV:] = f32(inputs["bq"])[pq]
    biases[1, 0:DIM] = f32(inputs["bo"])

    wqkv = np.concatenate([wk, wv, wq], 0)         # [3072, 2048] (k|v|q)
    shared = {
        "wqkvT": np.ascontiguousarray(wqkv.T).astype(BF),
        "woT": np.ascontiguousarray(wo.T).astype(BF),
        "biases": biases.astype(BF),
    }
    in_maps = []
    for c in range(NCORES):
        sl, sh = c, NCH - 1 - c
        pos = np.concatenate([np.arange(sl * 128, (sl + 1) * 128),
                              np.arange(sh * 128, (sh + 1) * 128)])
        cos, sin = _rope_tabs(pos)           # [64, 256] feature-major
        # row-major per-slot tables tiled 4x along free: [2 slot, 2 (cos,sin), 128, 256]
        rt = np.zeros((2, 2, 128, 256), np.float32)
        for slot_i in range(2):
            cr = cos[:, slot_i * 128:(slot_i + 1) * 128].T    # [128, 64]
            sr = sin[:, slot_i * 128:(slot_i + 1) * 128].T
            rt[slot_i, 0] = np.tile(cr, (1, 4))
            rt[slot_i, 1] = np.tile(sr, (1, 4))
        # masks [6, 128, 512]: 4 pair-tiles (units 0..7, [L|H0|L|H0]) then
        # 2 quad-tiles (units 8..15, H-stripe only)
        m = np.zeros((6, 128, 512), np.float32)
        for p_ in range(4):
            m[p_, :, 0:128] = _mask_for(sl, 2 * p_)
            m[p_, :, 256:384] = _mask_for(sl, 2 * p_ + 1)
            # H-stripe columns of pair units: chunks 0..7 are always visible
            m[p_, :, 128:256] = 1.0
            m[p_, :, 384:512] = 1.0
        for q_ in range(2):
            for u_ in range(4):
                m[4 + q_, :, u_ * 128:(u_ + 1) * 128] = _mask_for(sh, 8 + q_ * 4 + u_)
        im = {
            "x_rows": np.stack([x[sl * 128:(sl + 1) * 128],
                                x[sh * 128:(sh + 1) * 128]]),
            "rtab": rt,
            "masks": m.astype(BF),
            "biases": shared["biases"],
            "wqkvT": shared["wqkvT"], "woT": shared["woT"],
            "w1S": _wimg(w1[c * FSH:(c + 1) * FSH]),
            "w3S": _wimg(w3[c * FSH:(c + 1) * FSH]),
            "w2T": np.ascontiguousarray(
                w2[:, c * FSH:(c + 1) * FSH].T).astype(BF),
        }
        in_maps.append(im)
    return in_maps


def kernel(**inputs) -> np.ndarray:
    nc = _get_nc()
    in_maps = _prep_in_maps(inputs)
    res = run_bass_kernel_spmd(nc, in_maps, core_ids=list(range(NCORES)))
    out = np.empty((1, N, DIM), np.float32)
    for c in range(NCORES):
        o = res.results[c]["out"]            # [2, 128, DIM]
        out[0, c * 128:(c + 1) * 128] = o[0]
        out[0, (NCH - 1 - c) * 128:(NCH - c) * 128] = o[1]
    return out

